# revision 1
# baseline (speedup 1.0000x reference)
"""CloAttention Trainium2 Bass kernel.

Full inputs -> data-parallel over batch across 8 NeuronCores (4 images each)
-> full output.  All matmuls run on the PE in float32r/fp16 (1 cycle/row);
the 3x3 depthwise conv runs as 9 diagonal-matmul accumulations into PSUM in
fp16.
"""

import numpy as np
from contextlib import ExitStack

import concourse.bacc as bacc
import concourse.bass as bass
import concourse.tile as tile
from concourse import mybir
from concourse.bass_utils import run_bass_kernel_spmd

F32 = mybir.dt.float32
F32R = mybir.dt.float32r
F16 = mybir.dt.float16
AF = mybir.ActivationFunctionType
OP = mybir.AluOpType

N_CORES = 8
B_FULL = 32
B = B_FULL // N_CORES          # images per core
C = 256
H = W = 56
HW = H * W                     # 3136
PW = H + 2                     # 58 padded
NT = 7                         # pixel tiles per image
TS = HW // NT                  # 448 = 8 rows of 56
RPT = H // NT                  # 8 rows per tile
HEAD_DIM = 32
SCALER = HEAD_DIM ** -0.5
WIN = 7
HP = H // WIN                  # 8
POOL_N = HP * HP               # 64

ALL_STAGES = ("qkv", "pool", "gq", "dw", "attn", "den", "rec", "av", "proj")


def _body(ctx, tc, d, n_img=B, stages=ALL_STAGES):
    nc = tc.nc

    # ---------------- persistent weights ----------------
    wpool = ctx.enter_context(tc.tile_pool(name="wpool", bufs=1))

    def load_w(name, shape, dtype, src):
        t = wpool.tile(shape, dtype, tag=name, name=name)
        if dtype == F32R:
            nc.sync.dma_start(out=t, in_=src.bitcast(F32R))
        else:
            nc.sync.dma_start(out=t, in_=src)
        return t

    wqkv = [load_w(f"wqkv{c}", [128, 384], F16, d["wqkv"][c]) for c in range(2)]
    dwdiag = load_w("dwdiag", [128, 27 * 128], F16, d["dwdiag"])

    def dw_lhsT(cc, tap):
        i = cc * 9 + tap
        return dwdiag[:, i * 128:(i + 1) * 128]

    wact1 = load_w("wact1", [128, 128], F32R, d["wact1"])
    wact2 = load_w("wact2", [128, 128], F32R, d["wact2"])
    wgq = [load_w(f"wgq{c}", [128, 128], F16, d["wgq"][c]) for c in range(2)]
    wgkv = [load_w(f"wgkv{c}", [128, 256], F32R, d["wgkv"][c]) for c in range(2)]
    wproj = [load_w(f"wproj{c}", [128, 256], F16, d["wproj"][c]) for c in range(2)]
    denmask = [load_w(f"denmask{p}", [128, 128], F16, d["denmask"][p])
               for p in range(2)]
    bias_q = load_w("bias_q", [128, 1], F32, d["dwb"][0])
    bias_k = load_w("bias_k", [128, 1], F32, d["dwb"][1])
    bias_v = load_w("bias_v", [128, 1], F32, d["dwb"][2])
    bact1 = load_w("bact1", [128, 1], F32, d["bact1"])
    bact2 = load_w("bact2", [128, 1], F32, d["bact2"])

    # padded z buffers, x2 for image parity (borders stay zero; interiors
    # rewritten per image)
    zbufs = []
    for par in range(2):
        zs = [wpool.tile([128, PW * PW], F16, tag=f"z{j}_{par}",
                         name=f"z{j}_{par}") for j in range(3)]
        for z in zs:
            zg = z.rearrange("p (r c) -> p r c", c=PW)
            nc.vector.memset(zg[:, 0, :], 0.0)          # top border row
            nc.vector.memset(zg[:, PW - 1, :], 0.0)     # bottom border row
            nc.vector.memset(zg[:, :, 0], 0.0)          # left border col
            nc.vector.memset(zg[:, :, PW - 1], 0.0)     # right border col
        zbufs.append(zs)

    # ---------------- pools ----------------
    ps = ctx.enter_context(tc.tile_pool(name="ps", bufs=4, space="PSUM"))
    xpool = ctx.enter_context(tc.tile_pool(name="xpool", bufs=4))
    big = ctx.enter_context(tc.tile_pool(name="big", bufs=1))
    sm = ctx.enter_context(tc.tile_pool(name="sm", bufs=3))
    tiny = ctx.enter_context(tc.tile_pool(name="tiny", bufs=2))

    gq_sb = big.tile([128, HW], F16, tag="gq_sb")
    exp_sb = [big.tile([128, HW], F16, tag=f"exp{p}", name=f"exp{p}")
              for p in range(2)]
    rec_rep = big.tile([128, HW], F32, tag="rec_rep")
    cat_hi2 = [big.tile([128, HW], F16, tag=f"cat_hi{i}", name=f"cat_hi{i}")
               for i in range(2)]
    cat_lo2 = [big.tile([128, HW], F16, tag=f"cat_lo{i}", name=f"cat_lo{i}")
               for i in range(2)]
    if set(stages) != set(ALL_STAGES):
        # stage-masked debug builds read buffers their producer stage skipped
        for buf in (gq_sb, exp_sb[0], exp_sb[1], rec_rep,
                    *cat_hi2, *cat_lo2):
            nc.vector.memset(buf, 0.0)

    zgrid = {id(z): z.rearrange("p (r c) -> p r c", c=PW)
             for zs in zbufs for z in zs}

    def zwin(z, t, dy, dx):
        r0 = RPT * t + dy
        return zgrid[id(z)][:, r0:r0 + RPT, dx:dx + W]

    def zint(z, t):
        r0 = RPT * t + 1
        return zgrid[id(z)][:, r0:r0 + RPT, 1:1 + W]

    for b in range(n_img):
        z_q, z_k, z_v = zbufs[b % 2]
        cat_hi = cat_hi2[b % 2]
        cat_lo = cat_lo2[b % 2]
        # ---- load x ----
        x_sb = [xpool.tile([128, HW], F16, tag="x_sb", name="x_sb")
                for _ in range(2)]
        for cc in range(2):
            nc.sync.dma_start(out=x_sb[cc], in_=d["x"][b, cc])

        if "qkv" in stages:
            # qkv conv 256->384, evacuate into padded z (fp16)
            for t in range(NT):
                for j, (z, eng) in enumerate(
                        ((z_q, "act"), (z_k, "act"), (z_v, "dve"))):
                    pq = ps.tile([128, TS], F32, tag="psa", name="pq")
                    for cc in range(2):
                        nc.tensor.matmul(
                            pq[:], wqkv[cc][:, j * 128:(j + 1) * 128],
                            x_sb[cc][:, t * TS:(t + 1) * TS],
                            start=(cc == 0), stop=(cc == 1))
                    if eng == "act":
                        nc.scalar.copy(out=zint(z, t), in_=pq[:])
                    else:
                        nc.vector.tensor_copy(out=zint(z, t), in_=pq[:])

        if "pool" in stages:
            # pooling (sum over 7x7; 1/49 folded into wgkv)
            pooled = []
            for cc in range(2):
                pr1 = sm.tile([128, H * HP], F32, tag="pr1", name="pr1")
                nc.vector.tensor_reduce(
                    out=pr1.rearrange("p (y g) -> p y g", g=HP),
                    in_=x_sb[cc].rearrange(
                        "p (y g x) -> p y g x", y=H, g=HP),
                    axis=mybir.AxisListType.X, op=OP.add)
                po = tiny.tile([128, POOL_N], F32R, tag="pooled", name="po")
                with nc.allow_low_precision(reason="f32r is full-width fp32"):
                    nc.vector.tensor_reduce(
                        out=po.rearrange("p (a b) -> p a b", a=HP),
                        in_=pr1.rearrange("p (hp dy wp) -> p hp wp dy",
                                          hp=HP, dy=WIN),
                        axis=mybir.AxisListType.X, op=OP.add)
                pooled.append(po)

            # global kv: gk padded per head (other head half zero) so a head
            # pair accumulates into one [128, TS] PSUM tile at base 0
            pgk = ps.tile([128, POOL_N], F32, tag="psb", name="pgk")
            for cc in range(2):
                nc.tensor.matmul(pgk[:], wgkv[cc][:, 0:128], pooled[cc][:],
                                 start=(cc == 0), stop=(cc == 1))
            # per-head full-K lhsT: only rows 32h..32h+32 (head h's dims)
            # are nonzero, key cols at 64*(h%2); K=128 base-0 matmuls then
            # need no tile_position at all
            gk_pad = tiny.tile([128, 4 * 128], F16, tag="gk_pad")
            nc.vector.memset(gk_pad, 0.0)
            for h in range(4):
                nc.scalar.copy(
                    out=gk_pad[32 * h:32 * h + 32,
                               128 * h + 64 * (h % 2):
                               128 * h + 64 * (h % 2) + 64],
                    in_=pgk[32 * h:32 * h + 32, :])
            # gv transposed: [64 pos, 128 ch] via operand swap
            pgv = ps.tile([POOL_N, 128], F32, tag="psb", name="pgv")
            for cc in range(2):
                nc.tensor.matmul(pgv[:], pooled[cc][:], wgkv[cc][:, 128:256],
                                 start=(cc == 0), stop=(cc == 1))
            gvT = tiny.tile([POOL_N, 128], F16, tag="gvT")
            nc.scalar.copy(out=gvT[:], in_=pgv[:])

            # AV lhsT blocks, full-width with zero cols so the AV matmul
            # pair writes every PSUM row (no stale has_written)
            av0 = tiny.tile([128, 128], F16, tag="av0")
            av1 = tiny.tile([128, 128], F16, tag="av1")
            nc.vector.memset(av0, 0.0)
            nc.vector.memset(av1, 0.0)
            nc.vector.tensor_copy(out=av0[0:64, 0:32], in_=gvT[:, 0:32])
            nc.sync.dma_start(out=av0[64:128, 32:64], in_=gvT[:, 32:64])
            nc.vector.tensor_copy(out=av1[0:64, 64:96], in_=gvT[:, 64:96])
            nc.sync.dma_start(out=av1[64:128, 96:128], in_=gvT[:, 96:128])

        if "gq" in stages:
            for t in range(NT):
                pg = ps.tile([128, TS], F32, tag="psa", name="pg")
                for cc in range(2):
                    nc.tensor.matmul(pg[:], wgq[cc][:],
                                     x_sb[cc][:, t * TS:(t + 1) * TS],
                                     start=(cc == 0), stop=(cc == 1))
                nc.vector.tensor_copy(out=gq_sb[:, t * TS:(t + 1) * TS],
                                      in_=pg[:])

        if "dw" in stages:
            # local branch: dwconv + gating
            for t in range(NT):
                sl = slice(t * TS, (t + 1) * TS)
                pdq = ps.tile([128, TS], F32, tag="psa", name="pdq")
                for tap in range(9):
                    dy, dx = divmod(tap, 3)
                    nc.tensor.matmul(pdq[:], dw_lhsT(0, tap),
                                     zwin(z_q, t, dy, dx),
                                     start=(tap == 0), stop=(tap == 8))
                q_t = sm.tile([128, TS], F16, tag="q_t", name="q_t")
                nc.scalar.activation(out=q_t[:], in_=pdq[:], func=AF.Identity,
                                     bias=bias_q[:])
                pdk = ps.tile([128, TS], F32, tag="psa", name="pdk")
                for tap in range(9):
                    dy, dx = divmod(tap, 3)
                    nc.tensor.matmul(pdk[:], dw_lhsT(1, tap),
                                     zwin(z_k, t, dy, dx),
                                     start=(tap == 0), stop=(tap == 8))
                qk_t = sm.tile([128, TS], F32R, tag="qk_t", name="qk_t")
                nc.vector.scalar_tensor_tensor(
                    out=qk_t[:], in0=pdk[:], scalar=bias_k[:], in1=q_t[:],
                    op0=OP.add, op1=OP.mult)
                pa1 = ps.tile([128, TS], F32, tag="psa", name="pa1")
                nc.tensor.matmul(pa1[:], wact1[:], qk_t[:],
                                 start=True, stop=True)
                t_a = sm.tile([128, TS], F32, tag="t_a", name="t_a")
                nc.scalar.activation(out=t_a[:], in_=pa1[:], func=AF.Identity,
                                     bias=bact1[:])
                u_t = sm.tile([128, TS], F32, tag="u_t", name="u_t")
                nc.gpsimd.tensor_scalar(out=u_t[:], in0=t_a[:], scalar1=3.0,
                                        scalar2=0.0, op0=OP.add, op1=OP.max)
                hs_t = sm.tile([128, TS], F32R, tag="hs_t", name="hs_t")
                nc.vector.scalar_tensor_tensor(
                    out=hs_t[:], in0=u_t[:], scalar=6.0, in1=t_a[:],
                    op0=OP.min, op1=OP.mult)
                pa2 = ps.tile([128, TS], F32, tag="psa", name="pa2")
                nc.tensor.matmul(pa2[:], wact2[:], hs_t[:],
                                 start=True, stop=True)
                g_t = sm.tile([128, TS], F32, tag="g_t", name="g_t")
                nc.scalar.activation(out=g_t[:], in_=pa2[:], func=AF.Tanh,
                                     bias=bact2[:])
                pdv = ps.tile([128, TS], F32, tag="psa", name="pdv")
                for tap in range(9):
                    dy, dx = divmod(tap, 3)
                    nc.tensor.matmul(pdv[:], dw_lhsT(2, tap),
                                     zwin(z_v, t, dy, dx),
                                     start=(tap == 0), stop=(tap == 8))
                nc.vector.scalar_tensor_tensor(
                    out=cat_hi[:, sl], in0=pdv[:], scalar=bias_v[:],
                    in1=g_t[:], op0=OP.add, op1=OP.mult)

        if "attn" in stages:
            for t in range(NT):
                sl = slice(t * TS, (t + 1) * TS)
                pat = [ps.tile([128, TS], F32, tag="psb", name="pat")
                       for _ in range(2)]
                for h in range(4):
                    nc.tensor.matmul(
                        pat[h // 2][:], gk_pad[:, 128 * h:128 * h + 128],
                        gq_sb[:, sl],
                        start=(h % 2 == 0), stop=(h % 2 == 1))
                for p in range(2):
                    nc.scalar.activation(out=exp_sb[p][:, sl], in_=pat[p][:],
                                         func=AF.Exp, scale=float(SCALER))
                if "den" in stages:
                    pden = ps.tile([128, TS], F32, tag="psb", name="pden")
                    for p in range(2):
                        nc.tensor.matmul(pden[:], denmask[p][:],
                                         exp_sb[p][:, sl],
                                         start=(p == 0), stop=(p == 1))
                    if "rec" in stages:
                        nc.vector.reciprocal_approx_fast(out=rec_rep[:, sl],
                                                         in_=pden[:])
                    else:
                        nc.vector.tensor_copy(out=rec_rep[:, sl],
                                              in_=pden[:])

        if "av" in stages:
            for t in range(NT):
                sl = slice(t * TS, (t + 1) * TS)
                pav = ps.tile([128, TS], F32, tag="psb", name="pav")
                nc.tensor.matmul(pav[:], av0[:], exp_sb[0][:, sl],
                                 start=True, stop=False)
                nc.tensor.matmul(pav[:], av1[:], exp_sb[1][:, sl],
                                 start=False, stop=True)
                nc.vector.scalar_tensor_tensor(
                    out=cat_lo[:, sl], in0=pav[:], scalar=1.0,
                    in1=rec_rep[:, sl], op0=OP.mult, op1=OP.mult)

        if "proj" in stages:
            for t in range(NT):
                sl = slice(t * TS, (t + 1) * TS)
                for m in range(2):
                    pp = ps.tile([128, TS], F32, tag="psb", name="pp")
                    nc.tensor.matmul(pp[:],
                                     wproj[0][:, m * 128:(m + 1) * 128],
                                     cat_hi[:, sl], start=True, stop=False)
                    nc.tensor.matmul(pp[:],
                                     wproj[1][:, m * 128:(m + 1) * 128],
                                     cat_lo[:, sl], start=False, stop=True)
                    o_t = sm.tile([128, TS], F32, tag=f"o_t{m}",
                                  name=f"o_t{m}")
                    if m == 0:
                        nc.scalar.copy(out=o_t[:], in_=pp[:])
                    else:
                        nc.vector.tensor_copy(out=o_t[:], in_=pp[:])
                    nc.sync.dma_start(out=d["out"][b, m, :, sl], in_=o_t[:])


def _build(n_img=B, stages=ALL_STAGES):
    nc = bacc.Bacc("TRN2", target_bir_lowering=False, debug=False,
                   num_devices=N_CORES)
    dt = nc.dram_tensor
    d = {
        "x": dt("x", [B, 2, 128, HW], F16, kind="ExternalInput").ap(),
        "wqkv": dt("wqkv", [2, 128, 384], F16, kind="ExternalInput").ap(),
        "dwdiag": dt("dwdiag", [128, 27 * 128], F16,
                     kind="ExternalInput").ap(),
        "dwb": dt("dwb", [3, 128, 1], F32, kind="ExternalInput").ap(),
        "wact1": dt("wact1", [128, 128], F32, kind="ExternalInput").ap(),
        "bact1": dt("bact1", [128, 1], F32, kind="ExternalInput").ap(),
        "wact2": dt("wact2", [128, 128], F32, kind="ExternalInput").ap(),
        "bact2": dt("bact2", [128, 1], F32, kind="ExternalInput").ap(),
        "wgq": dt("wgq", [2, 128, 128], F16, kind="ExternalInput").ap(),
        "wgkv": dt("wgkv", [2, 128, 256], F32, kind="ExternalInput").ap(),
        "wproj": dt("wproj", [2, 128, 256], F16, kind="ExternalInput").ap(),
        "denmask": dt("denmask", [2, 128, 128], F16,
                      kind="ExternalInput").ap(),
        "out": dt("out", [B, 2, 128, HW], F32, kind="ExternalOutput").ap(),
    }
    with tile.TileContext(nc) as tc, ExitStack() as ctx:
        _body(ctx, tc, d, n_img=n_img, stages=stages)
    nc.compile()
    return nc


_NC = None


def _prep_weights(qkv_w, dw_w, dw_b, act1_w, act1_b, act2_w, act2_b,
                  gq_w, gkv_w, proj_w):
    f32 = np.float32
    w = {}
    w["wqkv"] = np.ascontiguousarray(
        qkv_w.T.reshape(2, 128, 384).astype(np.float16))
    taps = dw_w.reshape(384, 9)            # [c, tap]
    dwd = np.zeros((3, 9, 128, 128), dtype=np.float16)
    idx = np.arange(128)
    for cc in range(3):
        for tp in range(9):
            dwd[cc, tp, idx, idx] = taps[cc * 128:(cc + 1) * 128, tp]
    w["dwdiag"] = np.ascontiguousarray(
        dwd.transpose(2, 0, 1, 3).reshape(128, 27 * 128))
    w["dwb"] = dw_b.reshape(3, 128, 1).astype(f32)
    sc = np.float32(HEAD_DIM ** -0.5)
    w["wact1"] = np.ascontiguousarray((act1_w * sc).T.astype(f32))
    w["bact1"] = act1_b.reshape(128, 1).astype(f32)
    w["wact2"] = np.ascontiguousarray((act2_w / 6.0).T.astype(f32))
    w["bact2"] = act2_b.reshape(128, 1).astype(f32)
    w["wgq"] = np.ascontiguousarray(gq_w.T.reshape(2, 128, 128).astype(np.float16))
    w["wgkv"] = np.ascontiguousarray(
        (gkv_w / 49.0).T.reshape(2, 128, 256).astype(f32))
    w["wproj"] = np.ascontiguousarray(
        proj_w.T.reshape(2, 128, 256).astype(np.float16))
    dm = np.zeros((2, 128, 128), dtype=np.float16)
    for p in range(2):
        for hl in range(2):
            head = 2 * p + hl
            dm[p, 64 * hl:64 * hl + 64, 32 * head:32 * head + 32] = 1.0
    w["denmask"] = dm
    return w


def kernel(**inputs):
    global _NC
    x = inputs["x"]
    w = _prep_weights(
        inputs["qkv_w"], inputs["dw_w"], inputs["dw_b"],
        inputs["act1_w"], inputs["act1_b"], inputs["act2_w"],
        inputs["act2_b"], inputs["gq_w"], inputs["gkv_w"], inputs["proj_w"])
    if _NC is None:
        _NC = _build()
    in_maps = []
    for core in range(N_CORES):
        m = dict(w)
        m["x"] = np.ascontiguousarray(
            x[core * B:(core + 1) * B].reshape(B, 2, 128, HW)
            .astype(np.float16))
        in_maps.append(m)
    res = run_bass_kernel_spmd(_NC, in_maps, core_ids=list(range(N_CORES)))
    out = np.concatenate([r["out"] for r in res.results], axis=0)
    return out.reshape(B_FULL, C, H, W)



# revision 2
# speedup vs baseline: 1.9590x; 1.9590x over previous
"""CloAttention Trainium2 Bass kernel.

Full inputs -> data-parallel over batch across 8 NeuronCores (4 images each)
-> full output.  All matmuls run on the PE in fp16 (1 cycle/row); the 3x3
depthwise conv runs as 9 diagonal-matmul accumulations into PSUM.

Schedule: software-pipelined per image.  Loop A runs the depthwise/gating
chain with a 2-tile skew so the PE never waits on the scalar/vector chain;
loop B runs attention + projection for image b interleaved with the qkv/gq/
pool front-end of image b+1.  Hardswish clamp runs on DVE (tensor_scalar),
not GPSIMD, keeping the PE HAM clock-gate warm.
"""

import numpy as np
from contextlib import ExitStack

import concourse.bacc as bacc
import concourse.bass as bass
import concourse.tile as tile
from concourse import mybir
from concourse.bass_utils import run_bass_kernel_spmd

F32 = mybir.dt.float32
F16 = mybir.dt.float16
AF = mybir.ActivationFunctionType
OP = mybir.AluOpType

N_CORES = 8
B_FULL = 32
B = B_FULL // N_CORES          # images per core
C = 256
H = W = 56
HW = H * W                     # 3136
PW = H + 2                     # 58 padded
NT = 7                         # pixel tiles per image
TS = HW // NT                  # 448 = 8 rows of 56
RPT = H // NT                  # 8 rows per tile
HEAD_DIM = 32
SCALER = HEAD_DIM ** -0.5
WIN = 7
HP = H // WIN                  # 8
POOL_N = HP * HP               # 64


def _body(ctx, tc, d, n_img=B):
    nc = tc.nc

    # ---------------- persistent weights ----------------
    wpool = ctx.enter_context(tc.tile_pool(name="wpool", bufs=1))

    def load_w(name, shape, dtype, src):
        t = wpool.tile(shape, dtype, tag=name, name=name)
        nc.sync.dma_start(out=t, in_=src)
        return t

    wqkv = [load_w(f"wqkv{c}", [128, 384], F16, d["wqkv"][c]) for c in range(2)]
    dwdiag = load_w("dwdiag", [128, 27 * 128], F16, d["dwdiag"])

    def dw_lhsT(cc, tap):
        i = cc * 9 + tap
        return dwdiag[:, i * 128:(i + 1) * 128]

    wact1 = load_w("wact1", [128, 128], F16, d["wact1"])
    wact2 = load_w("wact2", [128, 128], F16, d["wact2"])
    wgq = [load_w(f"wgq{c}", [128, 128], F16, d["wgq"][c]) for c in range(2)]
    wgkv = [load_w(f"wgkv{c}", [128, 256], F16, d["wgkv"][c]) for c in range(2)]
    wproj = [load_w(f"wproj{c}", [128, 256], F16, d["wproj"][c]) for c in range(2)]
    denmask = [load_w(f"denmask{p}", [128, 128], F16, d["denmask"][p])
               for p in range(2)]
    bias_q = load_w("bias_q", [128, 1], F32, d["dwb"][0])
    bias_k = load_w("bias_k", [128, 1], F32, d["dwb"][1])
    bias_v = load_w("bias_v", [128, 1], F32, d["dwb"][2])
    bact1 = load_w("bact1", [128, 1], F32, d["bact1"])
    bact2 = load_w("bact2", [128, 1], F32, d["bact2"])

    # padded z buffers, x2 for image parity (borders stay zero; interiors
    # rewritten per image)
    zbufs = []
    for par in range(2):
        zs = [wpool.tile([128, PW * PW], F16, tag=f"z{j}_{par}",
                         name=f"z{j}_{par}") for j in range(3)]
        for z in zs:
            zg = z.rearrange("p (r c) -> p r c", c=PW)
            nc.vector.memset(zg[:, 0, :], 0.0)          # top border row
            nc.vector.memset(zg[:, PW - 1, :], 0.0)     # bottom border row
            nc.vector.memset(zg[:, :, 0], 0.0)          # left border col
            nc.vector.memset(zg[:, :, PW - 1], 0.0)     # right border col
        zbufs.append(zs)

    # block-diagonal gk (2 heads per matmul at K=128) and zero-padded AV
    # lhsT blocks, x2 parity; the zero regions are never rewritten so a
    # single memset at start suffices
    gk2 = []
    av_lhs = []
    for par in range(2):
        g = [wpool.tile([128, 128], F16, tag=f"gk2_{p}_{par}",
                        name=f"gk2_{p}_{par}") for p in range(2)]
        a = [wpool.tile([128, 128], F16, tag=f"av_{p}_{par}",
                        name=f"av_{p}_{par}") for p in range(2)]
        for tbuf in (*g, *a):
            nc.vector.memset(tbuf, 0.0)
        gk2.append(g)
        av_lhs.append(a)

    # ---------------- pools ----------------
    ps = ctx.enter_context(tc.tile_pool(name="ps", bufs=4, space="PSUM"))
    xpool = ctx.enter_context(tc.tile_pool(name="xpool", bufs=4))
    big = ctx.enter_context(tc.tile_pool(name="big", bufs=1))
    sm = ctx.enter_context(tc.tile_pool(name="sm", bufs=3))
    tiny = ctx.enter_context(tc.tile_pool(name="tiny", bufs=2))

    gq_sb2 = [big.tile([128, HW], F16, tag=f"gq_sb{i}", name=f"gq_sb{i}")
              for i in range(2)]
    exp_sb = [big.tile([128, HW], F16, tag=f"exp{p}", name=f"exp{p}")
              for p in range(2)]
    rec_rep = big.tile([128, HW], F32, tag="rec_rep")
    cat_hi2 = [big.tile([128, HW], F16, tag=f"cat_hi{i}", name=f"cat_hi{i}")
               for i in range(2)]
    cat_lo2 = [big.tile([128, HW], F16, tag=f"cat_lo{i}", name=f"cat_lo{i}")
               for i in range(2)]

    zgrid = {id(z): z.rearrange("p (r c) -> p r c", c=PW)
             for zs in zbufs for z in zs}

    def zwin(z, t, dy, dx):
        r0 = RPT * t + dy
        return zgrid[id(z)][:, r0:r0 + RPT, dx:dx + W]

    def zint(z, t):
        r0 = RPT * t + 1
        return zgrid[id(z)][:, r0:r0 + RPT, 1:1 + W]

    # ---------------- stage helpers ----------------
    def load_x(b):
        x_sb = [xpool.tile([128, HW], F16, tag=f"x{cc}", name=f"x{cc}")
                for cc in range(2)]
        for cc in range(2):
            nc.sync.dma_start(out=x_sb[cc], in_=d["x"][b, cc])
        return x_sb

    def qkv_tile(b, t, x_sb):
        z_q, z_k, z_v = zbufs[b % 2]
        for j, (z, eng) in enumerate(
                ((z_q, "act"), (z_k, "act"), (z_v, "dve"))):
            pq = ps.tile([128, TS], F32, tag="py", name="pq")
            for cc in range(2):
                nc.tensor.matmul(
                    pq[:], wqkv[cc][:, j * 128:(j + 1) * 128],
                    x_sb[cc][:, t * TS:(t + 1) * TS],
                    start=(cc == 0), stop=(cc == 1))
            if eng == "act":
                nc.scalar.copy(out=zint(z, t), in_=pq[:])
            else:
                nc.vector.tensor_copy(out=zint(z, t), in_=pq[:])

    def gq_tile(b, t, x_sb):
        pg = ps.tile([128, TS], F32, tag="py", name="pg")
        for cc in range(2):
            nc.tensor.matmul(pg[:], wgq[cc][:],
                             x_sb[cc][:, t * TS:(t + 1) * TS],
                             start=(cc == 0), stop=(cc == 1))
        nc.vector.tensor_copy(out=gq_sb2[b % 2][:, t * TS:(t + 1) * TS],
                              in_=pg[:])

    def pool_stage(b, x_sb):
        par = b % 2
        pooled = []
        for cc in range(2):
            pr1 = sm.tile([128, H * HP], F16, tag="pr1", name="pr1")
            with nc.allow_low_precision(reason="pool sums fit fp16"):
                nc.vector.tensor_reduce(
                    out=pr1.rearrange("p (y g) -> p y g", g=HP),
                    in_=x_sb[cc].rearrange(
                        "p (y g x) -> p y g x", y=H, g=HP),
                    axis=mybir.AxisListType.X, op=OP.add)
                po = tiny.tile([128, POOL_N], F16, tag="po", name="po")
                nc.vector.tensor_reduce(
                    out=po.rearrange("p (a b) -> p a b", a=HP),
                    in_=pr1.rearrange("p (hp dy wp) -> p hp wp dy",
                                      hp=HP, dy=WIN),
                    axis=mybir.AxisListType.X, op=OP.add)
            pooled.append(po)

        # gk: [128ch, 64pos] -> block-diag lhsT per head pair
        pgk = ps.tile([128, POOL_N], F32, tag="py", name="pgk")
        for cc in range(2):
            nc.tensor.matmul(pgk[:], wgkv[cc][:, 0:128], pooled[cc][:],
                             start=(cc == 0), stop=(cc == 1))
        for p in range(2):
            for hl in range(2):
                h = 2 * p + hl
                nc.scalar.copy(
                    out=gk2[par][p][32 * h:32 * h + 32,
                                    64 * hl:64 * hl + 64],
                    in_=pgk[32 * h:32 * h + 32, :])
        # gv transposed: [64 pos, 128 ch] via operand swap
        pgv = ps.tile([POOL_N, 128], F32, tag="py", name="pgv")
        for cc in range(2):
            nc.tensor.matmul(pgv[:], pooled[cc][:], wgkv[cc][:, 128:256],
                             start=(cc == 0), stop=(cc == 1))
        gvT = tiny.tile([POOL_N, 128], F16, tag="gvT", name="gvT")
        nc.scalar.copy(out=gvT[:], in_=pgv[:])
        av0, av1 = av_lhs[par]
        nc.vector.tensor_copy(out=av0[0:64, 0:32], in_=gvT[:, 0:32])
        nc.sync.dma_start(out=av0[64:128, 32:64], in_=gvT[:, 32:64])
        nc.vector.tensor_copy(out=av1[0:64, 64:96], in_=gvT[:, 64:96])
        nc.sync.dma_start(out=av1[64:128, 96:128], in_=gvT[:, 96:128])

    def dw_mm(z, cc, t, psname):
        p = ps.tile([128, TS], F32, tag="px", name=psname)
        for tap in range(9):
            dy, dx = divmod(tap, 3)
            nc.tensor.matmul(p[:], dw_lhsT(cc, tap), zwin(z, t, dy, dx),
                             start=(tap == 0), stop=(tap == 8))
        return p

    # ---------------- pipelined loops ----------------
    def loop_a(b):
        """dwconv + gating chain, 2-tile skew."""
        z_q, z_k, z_v = zbufs[b % 2]
        cat_hi = cat_hi2[b % 2]
        qk_t = {}
        ta = {}
        hs = {}
        for i in range(NT + 2):
            if i < NT:
                t = i
                pdq = dw_mm(z_q, 0, t, "pdq")
                q_t = sm.tile([128, TS], F16, tag="q_t", name="q_t")
                nc.scalar.activation(out=q_t[:], in_=pdq[:],
                                     func=AF.Identity, bias=bias_q[:])
                pdk = dw_mm(z_k, 1, t, "pdk")
                qk = sm.tile([128, TS], F16, tag="qk_t", name="qk_t")
                with nc.allow_low_precision(reason="qk product fits fp16"):
                    nc.vector.scalar_tensor_tensor(
                        out=qk[:], in0=pdk[:], scalar=bias_k[:], in1=q_t[:],
                        op0=OP.add, op1=OP.mult)
                qk_t[t] = qk
            if 1 <= i <= NT:
                t = i - 1
                pa1 = ps.tile([128, TS], F32, tag="py", name="pa1")
                nc.tensor.matmul(pa1[:], wact1[:], qk_t[t][:],
                                 start=True, stop=True)
                t_a = sm.tile([128, TS], F16, tag="t_a", name="t_a")
                nc.scalar.activation(out=t_a[:], in_=pa1[:],
                                     func=AF.Identity, bias=bact1[:])
                u_t = sm.tile([128, TS], F16, tag="u_t", name="u_t")
                nc.vector.tensor_scalar(out=u_t[:], in0=t_a[:], scalar1=3.0,
                                        scalar2=0.0, op0=OP.add, op1=OP.max)
                h_t = sm.tile([128, TS], F16, tag="hs_t", name="hs_t")
                with nc.allow_low_precision(reason="hardswish fits fp16"):
                    nc.vector.scalar_tensor_tensor(
                        out=h_t[:], in0=u_t[:], scalar=6.0, in1=t_a[:],
                        op0=OP.min, op1=OP.mult)
                ta[t] = t_a
                hs[t] = h_t
            if 2 <= i:
                t = i - 2
                sl = slice(t * TS, (t + 1) * TS)
                pa2 = ps.tile([128, TS], F32, tag="py", name="pa2")
                nc.tensor.matmul(pa2[:], wact2[:], hs[t][:],
                                 start=True, stop=True)
                g_t = sm.tile([128, TS], F16, tag="g_t", name="g_t")
                nc.scalar.activation(out=g_t[:], in_=pa2[:], func=AF.Tanh,
                                     bias=bact2[:])
                pdv = dw_mm(z_v, 2, t, "pdv")
                v_t = sm.tile([128, TS], F16, tag="v_t", name="v_t")
                nc.scalar.activation(out=v_t[:], in_=pdv[:],
                                     func=AF.Identity, bias=bias_v[:])
                with nc.allow_low_precision(reason="gated out fits fp16"):
                    nc.vector.scalar_tensor_tensor(
                        out=cat_hi[:, sl], in0=v_t[:], scalar=1.0,
                        in1=g_t[:], op0=OP.mult, op1=OP.mult)

    def loop_b(b, x_next):
        """attention + projection for image b, interleaved with the
        qkv/gq/pool front-end of image b+1."""
        par = b % 2
        cat_hi = cat_hi2[par]
        cat_lo = cat_lo2[par]
        gq_sb = gq_sb2[par]
        av0, av1 = av_lhs[par]
        if x_next is not None:
            pool_stage(b + 1, x_next)
        for i in range(NT + 3):
            if i < NT:
                t = i
                sl = slice(t * TS, (t + 1) * TS)
                if x_next is not None:
                    qkv_tile(b + 1, t, x_next)
                    gq_tile(b + 1, t, x_next)
                for p in range(2):
                    pat = ps.tile([128, TS], F32, tag="px", name="pat")
                    nc.tensor.matmul(pat[:], gk2[par][p][:], gq_sb[:, sl],
                                     start=True, stop=True)
                    nc.scalar.activation(out=exp_sb[p][:, sl], in_=pat[:],
                                         func=AF.Exp, scale=float(SCALER))
            if 1 <= i <= NT:
                t = i - 1
                sl = slice(t * TS, (t + 1) * TS)
                pden = ps.tile([128, TS], F32, tag="px", name="pden")
                for p in range(2):
                    nc.tensor.matmul(pden[:], denmask[p][:],
                                     exp_sb[p][:, sl],
                                     start=(p == 0), stop=(p == 1))
                nc.vector.reciprocal_approx_fast(out=rec_rep[:, sl],
                                                 in_=pden[:])
            if 2 <= i <= NT + 1:
                t = i - 2
                sl = slice(t * TS, (t + 1) * TS)
                pav = ps.tile([128, TS], F32, tag="px", name="pav")
                nc.tensor.matmul(pav[:], av0[:], exp_sb[0][:, sl],
                                 start=True, stop=False)
                nc.tensor.matmul(pav[:], av1[:], exp_sb[1][:, sl],
                                 start=False, stop=True)
                with nc.allow_low_precision(reason="attn out fits fp16"):
                    nc.vector.scalar_tensor_tensor(
                        out=cat_lo[:, sl], in0=pav[:], scalar=1.0,
                        in1=rec_rep[:, sl], op0=OP.mult, op1=OP.mult)
            if 3 <= i:
                t = i - 3
                sl = slice(t * TS, (t + 1) * TS)
                for m in range(2):
                    pp = ps.tile([128, TS], F32, tag="py", name="pp")
                    nc.tensor.matmul(pp[:],
                                     wproj[0][:, m * 128:(m + 1) * 128],
                                     cat_hi[:, sl], start=True, stop=False)
                    nc.tensor.matmul(pp[:],
                                     wproj[1][:, m * 128:(m + 1) * 128],
                                     cat_lo[:, sl], start=False, stop=True)
                    o_t = sm.tile([128, TS], F16, tag=f"o_t{m}",
                                  name=f"o_t{m}")
                    if m == 0:
                        nc.scalar.copy(out=o_t[:], in_=pp[:])
                    else:
                        nc.vector.tensor_copy(out=o_t[:], in_=pp[:])
                    nc.sync.dma_start(out=d["out"][b, m, :, sl], in_=o_t)

    # ---------------- program ----------------
    x_cur = load_x(0)
    for t in range(NT):
        qkv_tile(0, t, x_cur)
        gq_tile(0, t, x_cur)
    pool_stage(0, x_cur)

    for b in range(n_img):
        x_next = load_x(b + 1) if b + 1 < n_img else None
        loop_a(b)
        loop_b(b, x_next)


def _build(n_img=B):
    nc = bacc.Bacc("TRN2", target_bir_lowering=False, debug=False,
                   num_devices=N_CORES)
    dt = nc.dram_tensor
    d = {
        "x": dt("x", [B, 2, 128, HW], F16, kind="ExternalInput").ap(),
        "wqkv": dt("wqkv", [2, 128, 384], F16, kind="ExternalInput").ap(),
        "dwdiag": dt("dwdiag", [128, 27 * 128], F16,
                     kind="ExternalInput").ap(),
        "dwb": dt("dwb", [3, 128, 1], F32, kind="ExternalInput").ap(),
        "wact1": dt("wact1", [128, 128], F16, kind="ExternalInput").ap(),
        "bact1": dt("bact1", [128, 1], F32, kind="ExternalInput").ap(),
        "wact2": dt("wact2", [128, 128], F16, kind="ExternalInput").ap(),
        "bact2": dt("bact2", [128, 1], F32, kind="ExternalInput").ap(),
        "wgq": dt("wgq", [2, 128, 128], F16, kind="ExternalInput").ap(),
        "wgkv": dt("wgkv", [2, 128, 256], F16, kind="ExternalInput").ap(),
        "wproj": dt("wproj", [2, 128, 256], F16, kind="ExternalInput").ap(),
        "denmask": dt("denmask", [2, 128, 128], F16,
                      kind="ExternalInput").ap(),
        "out": dt("out", [B, 2, 128, HW], F16, kind="ExternalOutput").ap(),
    }
    with tile.TileContext(nc) as tc, ExitStack() as ctx:
        _body(ctx, tc, d, n_img=n_img)
    nc.compile()
    return nc


_NC = None


def _prep_weights(qkv_w, dw_w, dw_b, act1_w, act1_b, act2_w, act2_b,
                  gq_w, gkv_w, proj_w):
    f32 = np.float32
    f16 = np.float16
    w = {}
    w["wqkv"] = np.ascontiguousarray(qkv_w.T.reshape(2, 128, 384).astype(f16))
    taps = dw_w.reshape(384, 9)            # [c, tap]
    dwd = np.zeros((3, 9, 128, 128), dtype=f16)
    idx = np.arange(128)
    for cc in range(3):
        for tp in range(9):
            dwd[cc, tp, idx, idx] = taps[cc * 128:(cc + 1) * 128, tp]
    w["dwdiag"] = np.ascontiguousarray(
        dwd.transpose(2, 0, 1, 3).reshape(128, 27 * 128))
    w["dwb"] = dw_b.reshape(3, 128, 1).astype(f32)
    sc = np.float32(HEAD_DIM ** -0.5)
    w["wact1"] = np.ascontiguousarray((act1_w * sc).T.astype(f16))
    w["bact1"] = act1_b.reshape(128, 1).astype(f32)
    w["wact2"] = np.ascontiguousarray((act2_w / 6.0).T.astype(f16))
    w["bact2"] = act2_b.reshape(128, 1).astype(f32)
    w["wgq"] = np.ascontiguousarray(gq_w.T.reshape(2, 128, 128).astype(f16))
    w["wgkv"] = np.ascontiguousarray(
        (gkv_w / 49.0).T.reshape(2, 128, 256).astype(f16))
    w["wproj"] = np.ascontiguousarray(
        proj_w.T.reshape(2, 128, 256).astype(f16))
    dm = np.zeros((2, 128, 128), dtype=f16)
    for p in range(2):
        for hl in range(2):
            head = 2 * p + hl
            dm[p, 64 * hl:64 * hl + 64, 32 * head:32 * head + 32] = 1.0
    w["denmask"] = dm
    return w


def kernel(**inputs):
    global _NC
    x = inputs["x"]
    w = _prep_weights(
        inputs["qkv_w"], inputs["dw_w"], inputs["dw_b"],
        inputs["act1_w"], inputs["act1_b"], inputs["act2_w"],
        inputs["act2_b"], inputs["gq_w"], inputs["gkv_w"], inputs["proj_w"])
    if _NC is None:
        _NC = _build()
    in_maps = []
    for core in range(N_CORES):
        m = dict(w)
        m["x"] = np.ascontiguousarray(
            x[core * B:(core + 1) * B].reshape(B, 2, 128, HW)
            .astype(np.float16))
        in_maps.append(m)
    res = run_bass_kernel_spmd(_NC, in_maps, core_ids=list(range(N_CORES)))
    out = np.concatenate([r["out"] for r in res.results], axis=0)
    return out.reshape(B_FULL, C, H, W).astype(np.float32)


# revision 5
# speedup vs baseline: 2.1300x; 1.0873x over previous
"""CloAttention Trainium2 Bass kernel.

Full inputs -> data-parallel over batch across 8 NeuronCores (4 images each)
-> full output.  All matmuls run on the PE in fp16 (1 cycle/row); the 3x3
depthwise conv runs as 9 diagonal-matmul accumulations into PSUM.

Schedule: software-pipelined per image.  Loop A runs the depthwise/gating
chain with a 2-tile skew so the PE never waits on the scalar/vector chain;
loop B runs attention + projection for image b interleaved with the qkv/gq
front-end of image b+1.  Pooling runs on the otherwise-idle GPSIMD engine;
a couple of dw-v tiles per image run as shifted multiply-adds on DVE to
shave PE work.  Weights arrive in two consolidated DMAs and dummy matmuls
warm the PE HAM clock-gate during the initial DMA wait.
"""

import numpy as np
from contextlib import ExitStack

import concourse.bacc as bacc
import concourse.bass as bass
import concourse.tile as tile
from concourse import mybir
from concourse.bass_utils import run_bass_kernel_spmd

F32 = mybir.dt.float32
F16 = mybir.dt.float16
AF = mybir.ActivationFunctionType
OP = mybir.AluOpType

N_CORES = 8
B_FULL = 32
B = B_FULL // N_CORES          # images per core
C = 256
H = W = 56
HW = H * W                     # 3136
PW = H + 2                     # 58 padded
NT = 7                         # pixel tiles per image
TS = HW // NT                  # 448 = 8 rows of 56
RPT = H // NT                  # 8 rows per tile
HEAD_DIM = 32
SCALER = HEAD_DIM ** -0.5
WIN = 7
HP = H // WIN                  # 8
POOL_N = HP * HP               # 64

OFFV = (1, 4)                  # tiles whose dw-v runs on DVE, not PE

# f16 weight block column offsets
WCOL = {}
_off = 0
for _nm, _w in (("wqkv0", 384), ("wqkv1", 384), ("dwdiag", 3456),
                ("wact1", 128), ("wact2", 128), ("wgq0", 128),
                ("wgq1", 128), ("wgkv0", 256), ("wgkv1", 256),
                ("wproj0", 256), ("wproj1", 256), ("denmask0", 128),
                ("denmask1", 128)):
    WCOL[_nm] = (_off, _off + _w)
    _off += _w
W16_COLS = _off                # 6016
W32_COLS = 14                  # dwb q,k,v | bact1 | bact2 | dwv taps 0..8


def _body(ctx, tc, d, n_img=B):
    nc = tc.nc

    # ---------------- persistent weights (2 consolidated DMAs) ----------
    wpool = ctx.enter_context(tc.tile_pool(name="wpool", bufs=1))

    warm_src = wpool.tile([128, 64], F16, tag="warm_src", name="warm_src")
    nc.vector.memset(warm_src, 0.0)

    wf16 = wpool.tile([128, W16_COLS], F16, tag="wf16", name="wf16")
    nc.sync.dma_start(out=wf16, in_=d["wf16"])
    wf32 = wpool.tile([128, W32_COLS], F32, tag="wf32", name="wf32")
    nc.sync.dma_start(out=wf32, in_=d["wf32"])

    def wv(name):
        a, b_ = WCOL[name]
        return wf16[:, a:b_]

    wqkv = [wv("wqkv0"), wv("wqkv1")]
    dwdiag = wv("dwdiag")
    wact1 = wv("wact1")
    wact2 = wv("wact2")
    wgq = [wv("wgq0"), wv("wgq1")]
    wgkv = [wv("wgkv0"), wv("wgkv1")]
    wproj = [wv("wproj0"), wv("wproj1")]
    denmask = [wv("denmask0"), wv("denmask1")]
    bias_q = wf32[:, 0:1]
    bias_k = wf32[:, 1:2]
    bias_v = wf32[:, 2:3]
    bact1 = wf32[:, 3:4]
    bact2 = wf32[:, 4:5]

    def wv_tap(tap):
        return wf32[:, 5 + tap:6 + tap]

    def dw_lhsT(cc, tap):
        i = cc * 9 + tap
        return dwdiag[:, i * 128:(i + 1) * 128]

    # padded z buffers, x2 for image parity (borders stay zero; interiors
    # rewritten per image)
    zbufs = []
    for par in range(2):
        zs = [wpool.tile([128, PW * PW], F16, tag=f"z{j}_{par}",
                         name=f"z{j}_{par}") for j in range(3)]
        for z in zs:
            zg = z.rearrange("p (r c) -> p r c", c=PW)
            nc.vector.memset(zg[:, 0, :], 0.0)          # top border row
            nc.vector.memset(zg[:, PW - 1, :], 0.0)     # bottom border row
            nc.vector.memset(zg[:, :, 0], 0.0)          # left border col
            nc.vector.memset(zg[:, :, PW - 1], 0.0)     # right border col
        zbufs.append(zs)

    # block-diagonal gk (2 heads per matmul at K=128) and zero-padded AV
    # lhsT blocks, x2 parity; zero regions never rewritten -> memset once
    gk2 = []
    av_lhs = []
    for par in range(2):
        g = [wpool.tile([128, 128], F16, tag=f"gk2_{p}_{par}",
                        name=f"gk2_{p}_{par}") for p in range(2)]
        a = [wpool.tile([128, 128], F16, tag=f"av_{p}_{par}",
                        name=f"av_{p}_{par}") for p in range(2)]
        for tbuf in (*g, *a):
            nc.vector.memset(tbuf, 0.0)
        gk2.append(g)
        av_lhs.append(a)

    # ---------------- pools ----------------
    ps = ctx.enter_context(tc.tile_pool(name="ps", bufs=4, space="PSUM"))
    xpool = ctx.enter_context(tc.tile_pool(name="xpool", bufs=4))
    big = ctx.enter_context(tc.tile_pool(name="big", bufs=1))
    sm = ctx.enter_context(tc.tile_pool(name="sm", bufs=3))
    tiny = ctx.enter_context(tc.tile_pool(name="tiny", bufs=2))

    gq_sb2 = [big.tile([128, HW], F16, tag=f"gq_sb{i}", name=f"gq_sb{i}")
              for i in range(2)]
    exp_sb = [big.tile([128, HW], F16, tag=f"exp{p}", name=f"exp{p}")
              for p in range(2)]
    rec_rep = big.tile([128, HW], F32, tag="rec_rep")
    cat_hi2 = [big.tile([128, HW], F16, tag=f"cat_hi{i}", name=f"cat_hi{i}")
               for i in range(2)]
    cat_lo2 = [big.tile([128, HW], F16, tag=f"cat_lo{i}", name=f"cat_lo{i}")
               for i in range(2)]

    zgrid = {id(z): z.rearrange("p (r c) -> p r c", c=PW)
             for zs in zbufs for z in zs}

    def zwin(z, t, dy, dx):
        r0 = RPT * t + dy
        return zgrid[id(z)][:, r0:r0 + RPT, dx:dx + W]

    def zint(z, t):
        r0 = RPT * t + 1
        return zgrid[id(z)][:, r0:r0 + RPT, 1:1 + W]

    # PE warmup: dummy matmuls keep the HAM clock-gate busy while the
    # weight/x DMAs land, so real matmuls start at 2.4 GHz
    for wi in range(48):
        pw = ps.tile([64, 64], F32, tag="py", name="pwarm")
        nc.tensor.matmul(pw[:], warm_src[:], warm_src[:],
                         start=True, stop=True)

    # ---------------- stage helpers ----------------
    def load_x(b):
        x_sb = [xpool.tile([128, HW], F16, tag=f"x{cc}", name=f"x{cc}")
                for cc in range(2)]
        for cc in range(2):
            nc.sync.dma_start(out=x_sb[cc], in_=d["x"][b, cc])
        return x_sb

    def qkv_tile(b, t, x_sb):
        z_q, z_k, z_v = zbufs[b % 2]
        for j, (z, eng) in enumerate(
                ((z_q, "act"), (z_k, "act"), (z_v, "dve"))):
            pq = ps.tile([128, TS], F32, tag="py", name="pq")
            for cc in range(2):
                nc.tensor.matmul(
                    pq[:], wqkv[cc][:, j * 128:(j + 1) * 128],
                    x_sb[cc][:, t * TS:(t + 1) * TS],
                    start=(cc == 0), stop=(cc == 1))
            if eng == "act":
                nc.scalar.copy(out=zint(z, t), in_=pq[:])
            else:
                nc.vector.tensor_copy(out=zint(z, t), in_=pq[:])

    def gq_tile(b, t, x_sb):
        pg = ps.tile([128, TS], F32, tag="py", name="pg")
        for cc in range(2):
            nc.tensor.matmul(pg[:], wgq[cc][:],
                             x_sb[cc][:, t * TS:(t + 1) * TS],
                             start=(cc == 0), stop=(cc == 1))
        nc.vector.tensor_copy(out=gq_sb2[b % 2][:, t * TS:(t + 1) * TS],
                              in_=pg[:])

    def pool_reduce(x_sb):
        """7x7 window sums; issued early so the results have slack."""
        pooled = []
        for cc in range(2):
            pr1 = sm.tile([128, H * HP], F32, tag="pr1", name="pr1")
            nc.vector.tensor_reduce(
                out=pr1.rearrange("p (y g) -> p y g", g=HP),
                in_=x_sb[cc].rearrange("p (y g x) -> p y g x", y=H, g=HP),
                axis=mybir.AxisListType.X, op=OP.add)
            po = tiny.tile([128, POOL_N], F16, tag="po", name="po")
            with nc.allow_low_precision(reason="pool sums fit fp16"):
                nc.vector.tensor_reduce(
                    out=po.rearrange("p (a b) -> p a b", a=HP),
                    in_=pr1.rearrange("p (hp dy wp) -> p hp wp dy",
                                      hp=HP, dy=WIN),
                    axis=mybir.AxisListType.X, op=OP.add)
            pooled.append(po)
        return pooled

    def pool_finish(b, pooled):
        """global-kv matmuls + lhsT packing for image b's attention."""
        par = b % 2
        pgk = ps.tile([128, POOL_N], F32, tag="py", name="pgk")
        for cc in range(2):
            nc.tensor.matmul(pgk[:], wgkv[cc][:, 0:128], pooled[cc][:],
                             start=(cc == 0), stop=(cc == 1))
        for p in range(2):
            for hl in range(2):
                h = 2 * p + hl
                nc.scalar.copy(
                    out=gk2[par][p][32 * h:32 * h + 32,
                                    64 * hl:64 * hl + 64],
                    in_=pgk[32 * h:32 * h + 32, :])
        pgv = ps.tile([POOL_N, 128], F32, tag="py", name="pgv")
        for cc in range(2):
            nc.tensor.matmul(pgv[:], pooled[cc][:], wgkv[cc][:, 128:256],
                             start=(cc == 0), stop=(cc == 1))
        gvT = tiny.tile([POOL_N, 128], F16, tag="gvT", name="gvT")
        nc.scalar.copy(out=gvT[:], in_=pgv[:])
        av0, av1 = av_lhs[par]
        nc.vector.tensor_copy(out=av0[0:64, 0:32], in_=gvT[:, 0:32])
        nc.sync.dma_start(out=av0[64:128, 32:64], in_=gvT[:, 32:64])
        nc.vector.tensor_copy(out=av1[0:64, 64:96], in_=gvT[:, 64:96])
        nc.sync.dma_start(out=av1[64:128, 96:128], in_=gvT[:, 96:128])

    def dw_mm(z, cc, t, psname):
        p = ps.tile([128, TS], F32, tag="px", name=psname)
        for tap in range(9):
            dy, dx = divmod(tap, 3)
            nc.tensor.matmul(p[:], dw_lhsT(cc, tap), zwin(z, t, dy, dx),
                             start=(tap == 0), stop=(tap == 8))
        return p

    def dwv_vector(z_v, t):
        """dw-v for one tile as 9 shifted multiply-adds on DVE; returns
        the accumulated (dwv + bias_v) tile in fp16."""
        acc = sm.tile([128, TS], F16, tag="accv", name="accv")
        with nc.allow_low_precision(reason="dwv fits fp16"):
            nc.vector.tensor_scalar(
                out=acc[:], in0=zwin(z_v, t, 0, 0), scalar1=wv_tap(0),
                scalar2=bias_v, op0=OP.mult, op1=OP.add)
            for tap in range(1, 9):
                dy, dx = divmod(tap, 3)
                nacc = sm.tile([128, TS], F16, tag="accv", name="accv")
                nc.vector.scalar_tensor_tensor(
                    out=nacc[:], in0=zwin(z_v, t, dy, dx),
                    scalar=wv_tap(tap), in1=acc[:],
                    op0=OP.mult, op1=OP.add)
                acc = nacc
        return acc

    # ---------------- pipelined loops ----------------
    def loop_a(b):
        """dwconv + gating chain, 2-tile skew."""
        z_q, z_k, z_v = zbufs[b % 2]
        cat_hi = cat_hi2[b % 2]
        qk_t = {}
        ta = {}
        hs = {}
        for i in range(NT + 2):
            if i < NT:
                t = i
                pdq = dw_mm(z_q, 0, t, "pdq")
                q_t = sm.tile([128, TS], F16, tag="q_t", name="q_t")
                nc.scalar.activation(out=q_t[:], in_=pdq[:],
                                     func=AF.Identity, bias=bias_q)
                pdk = dw_mm(z_k, 1, t, "pdk")
                qk = sm.tile([128, TS], F16, tag="qk_t", name="qk_t")
                with nc.allow_low_precision(reason="qk product fits fp16"):
                    nc.vector.scalar_tensor_tensor(
                        out=qk[:], in0=pdk[:], scalar=bias_k, in1=q_t[:],
                        op0=OP.add, op1=OP.mult)
                qk_t[t] = qk
            if 1 <= i <= NT:
                t = i - 1
                pa1 = ps.tile([128, TS], F32, tag="py", name="pa1")
                nc.tensor.matmul(pa1[:], wact1[:], qk_t[t][:],
                                 start=True, stop=True)
                t_a = sm.tile([128, TS], F16, tag="t_a", name="t_a")
                nc.scalar.activation(out=t_a[:], in_=pa1[:],
                                     func=AF.Identity, bias=bact1)
                u_t = sm.tile([128, TS], F16, tag="u_t", name="u_t")
                nc.vector.tensor_scalar(out=u_t[:], in0=t_a[:], scalar1=3.0,
                                        scalar2=0.0, op0=OP.add, op1=OP.max)
                h_t = sm.tile([128, TS], F16, tag="hs_t", name="hs_t")
                with nc.allow_low_precision(reason="hardswish fits fp16"):
                    nc.vector.scalar_tensor_tensor(
                        out=h_t[:], in0=u_t[:], scalar=6.0, in1=t_a[:],
                        op0=OP.min, op1=OP.mult)
                ta[t] = t_a
                hs[t] = h_t
            if 2 <= i:
                t = i - 2
                sl = slice(t * TS, (t + 1) * TS)
                pa2 = ps.tile([128, TS], F32, tag="py", name="pa2")
                nc.tensor.matmul(pa2[:], wact2[:], hs[t][:],
                                 start=True, stop=True)
                g_t = sm.tile([128, TS], F16, tag="g_t", name="g_t")
                nc.scalar.activation(out=g_t[:], in_=pa2[:], func=AF.Tanh,
                                     bias=bact2)
                if t in OFFV:
                    acc = dwv_vector(z_v, t)
                    with nc.allow_low_precision(reason="gated out fp16"):
                        nc.vector.scalar_tensor_tensor(
                            out=cat_hi[:, sl], in0=acc[:], scalar=1.0,
                            in1=g_t[:], op0=OP.mult, op1=OP.mult)
                else:
                    pdv = dw_mm(z_v, 2, t, "pdv")
                    v_t = sm.tile([128, TS], F16, tag="v_t", name="v_t")
                    nc.scalar.activation(out=v_t[:], in_=pdv[:],
                                         func=AF.Identity, bias=bias_v)
                    with nc.allow_low_precision(reason="gated out fp16"):
                        nc.vector.scalar_tensor_tensor(
                            out=cat_hi[:, sl], in0=v_t[:], scalar=1.0,
                            in1=g_t[:], op0=OP.mult, op1=OP.mult)

    def loop_b(b, x_next, pooled_next):
        """attention + projection for image b, interleaved with the
        qkv/gq front-end of image b+1."""
        par = b % 2
        cat_hi = cat_hi2[par]
        cat_lo = cat_lo2[par]
        gq_sb = gq_sb2[par]
        av0, av1 = av_lhs[par]
        for i in range(NT + 3):
            if i < NT:
                t = i
                sl = slice(t * TS, (t + 1) * TS)
                if x_next is not None:
                    qkv_tile(b + 1, t, x_next)
                    gq_tile(b + 1, t, x_next)
                for p in range(2):
                    pat = ps.tile([128, TS], F32, tag="px", name="pat")
                    nc.tensor.matmul(pat[:], gk2[par][p][:], gq_sb[:, sl],
                                     start=True, stop=True)
                    nc.scalar.activation(out=exp_sb[p][:, sl], in_=pat[:],
                                         func=AF.Exp, scale=float(SCALER))
            if 1 <= i <= NT:
                t = i - 1
                sl = slice(t * TS, (t + 1) * TS)
                pden = ps.tile([128, TS], F32, tag="px", name="pden")
                for p in range(2):
                    nc.tensor.matmul(pden[:], denmask[p][:],
                                     exp_sb[p][:, sl],
                                     start=(p == 0), stop=(p == 1))
                nc.vector.reciprocal_approx_fast(out=rec_rep[:, sl],
                                                 in_=pden[:])
            if 2 <= i <= NT + 1:
                t = i - 2
                sl = slice(t * TS, (t + 1) * TS)
                pav = ps.tile([128, TS], F32, tag="px", name="pav")
                nc.tensor.matmul(pav[:], av0[:], exp_sb[0][:, sl],
                                 start=True, stop=False)
                nc.tensor.matmul(pav[:], av1[:], exp_sb[1][:, sl],
                                 start=False, stop=True)
                with nc.allow_low_precision(reason="attn out fits fp16"):
                    nc.vector.scalar_tensor_tensor(
                        out=cat_lo[:, sl], in0=pav[:], scalar=1.0,
                        in1=rec_rep[:, sl], op0=OP.mult, op1=OP.mult)
            if 3 <= i:
                t = i - 3
                sl = slice(t * TS, (t + 1) * TS)
                for m in range(2):
                    pp = ps.tile([128, TS], F32, tag="py", name="pp")
                    nc.tensor.matmul(pp[:],
                                     wproj[0][:, m * 128:(m + 1) * 128],
                                     cat_hi[:, sl], start=True, stop=False)
                    nc.tensor.matmul(pp[:],
                                     wproj[1][:, m * 128:(m + 1) * 128],
                                     cat_lo[:, sl], start=False, stop=True)
                    o_t = sm.tile([128, TS], F16, tag=f"o_t{m}",
                                  name=f"o_t{m}")
                    if m == 0:
                        nc.scalar.copy(out=o_t[:], in_=pp[:])
                    else:
                        nc.vector.tensor_copy(out=o_t[:], in_=pp[:])
                    nc.sync.dma_start(out=d["out"][b, m, :, sl], in_=o_t)
        if pooled_next is not None:
            pool_finish(b + 1, pooled_next)

    # ---------------- program ----------------
    x_cur = load_x(0)
    pooled = pool_reduce(x_cur)
    for t in range(NT):
        qkv_tile(0, t, x_cur)
        gq_tile(0, t, x_cur)
    pool_finish(0, pooled)

    for b in range(n_img):
        if b + 1 < n_img:
            x_next = load_x(b + 1)
            pooled_next = pool_reduce(x_next)
        else:
            x_next = pooled_next = None
        loop_a(b)
        loop_b(b, x_next, pooled_next)


def _build(n_img=B):
    nc = bacc.Bacc("TRN2", target_bir_lowering=False, debug=False,
                   num_devices=N_CORES)
    dt = nc.dram_tensor
    d = {
        "x": dt("x", [B, 2, 128, HW], F16, kind="ExternalInput").ap(),
        "wf16": dt("wf16", [128, W16_COLS], F16, kind="ExternalInput").ap(),
        "wf32": dt("wf32", [128, W32_COLS], F32, kind="ExternalInput").ap(),
        "out": dt("out", [B, 2, 128, HW], F16, kind="ExternalOutput").ap(),
    }
    with tile.TileContext(nc) as tc, ExitStack() as ctx:
        _body(ctx, tc, d, n_img=n_img)
    nc.compile()
    return nc


_NC = None


def _prep_weights(qkv_w, dw_w, dw_b, act1_w, act1_b, act2_w, act2_b,
                  gq_w, gkv_w, proj_w):
    f32 = np.float32
    f16 = np.float16
    sc = np.float32(HEAD_DIM ** -0.5)

    wqkv = qkv_w.T.reshape(2, 128, 384).astype(f16)
    taps = dw_w.reshape(384, 9)            # [c, tap]
    dwd = np.zeros((3, 9, 128, 128), dtype=f16)
    idx = np.arange(128)
    for cc in range(3):
        for tp in range(9):
            dwd[cc, tp, idx, idx] = taps[cc * 128:(cc + 1) * 128, tp]
    dwdiag = dwd.transpose(2, 0, 1, 3).reshape(128, 27 * 128)
    wact1 = (act1_w * sc).T.astype(f16)
    wact2 = (act2_w / 6.0).T.astype(f16)
    wgq = gq_w.T.reshape(2, 128, 128).astype(f16)
    wgkv = (gkv_w / 49.0).T.reshape(2, 128, 256).astype(f16)
    wproj = proj_w.T.reshape(2, 128, 256).astype(f16)
    dm = np.zeros((2, 128, 128), dtype=f16)
    for p in range(2):
        for hl in range(2):
            head = 2 * p + hl
            dm[p, 64 * hl:64 * hl + 64, 32 * head:32 * head + 32] = 1.0

    blocks = {"wqkv0": wqkv[0], "wqkv1": wqkv[1], "dwdiag": dwdiag,
              "wact1": wact1, "wact2": wact2, "wgq0": wgq[0],
              "wgq1": wgq[1], "wgkv0": wgkv[0], "wgkv1": wgkv[1],
              "wproj0": wproj[0], "wproj1": wproj[1],
              "denmask0": dm[0], "denmask1": dm[1]}
    wf16 = np.zeros((128, W16_COLS), dtype=f16)
    for nm, (a, b_) in WCOL.items():
        wf16[:, a:b_] = blocks[nm]

    wf32 = np.zeros((128, W32_COLS), dtype=f32)
    wf32[:, 0:3] = dw_b.reshape(3, 128).T
    wf32[:, 3] = act1_b.astype(f32)
    wf32[:, 4] = act2_b.astype(f32)
    wf32[:, 5:14] = taps[256:384].astype(f32)   # dw-v taps for DVE path

    return {"wf16": np.ascontiguousarray(wf16),
            "wf32": np.ascontiguousarray(wf32)}


def kernel(**inputs):
    global _NC
    x = inputs["x"]
    w = _prep_weights(
        inputs["qkv_w"], inputs["dw_w"], inputs["dw_b"],
        inputs["act1_w"], inputs["act1_b"], inputs["act2_w"],
        inputs["act2_b"], inputs["gq_w"], inputs["gkv_w"], inputs["proj_w"])
    if _NC is None:
        _NC = _build()
    in_maps = []
    for core in range(N_CORES):
        m = dict(w)
        m["x"] = np.ascontiguousarray(
            x[core * B:(core + 1) * B].reshape(B, 2, 128, HW)
            .astype(np.float16))
        in_maps.append(m)
    res = run_bass_kernel_spmd(_NC, in_maps, core_ids=list(range(N_CORES)))
    out = np.concatenate([r["out"] for r in res.results], axis=0)
    return out.reshape(B_FULL, C, H, W).astype(np.float32)


# revision 15
# speedup vs baseline: 2.2991x; 1.0794x over previous
"""CloAttention Trainium2 Bass kernel.

Full inputs -> data-parallel over batch across 8 NeuronCores (4 images each)
-> full output.  All matmuls run on the PE in fp16 (1 cycle/row); the 3x3
depthwise conv runs as 9 diagonal-matmul accumulations into PSUM.

Schedule: software-pipelined per image.  Loop A runs the depthwise/gating
chain with a 2-tile skew so the PE never waits on the scalar/vector chain;
loop B runs attention + projection for image b interleaved with the qkv/gq
front-end of image b+1.  Pooling runs on the otherwise-idle GPSIMD engine;
a couple of dw-v tiles per image run as shifted multiply-adds on DVE to
shave PE work.  Weights arrive in two consolidated DMAs and dummy matmuls
warm the PE HAM clock-gate during the initial DMA wait.
"""

import numpy as np
from contextlib import ExitStack

import concourse.bacc as bacc
import concourse.bass as bass
import concourse.tile as tile
from concourse import mybir
from concourse.bass_utils import run_bass_kernel_spmd

F32 = mybir.dt.float32
F16 = mybir.dt.float16
F8 = mybir.dt.float8e4
AF = mybir.ActivationFunctionType
OP = mybir.AluOpType

# dw tap pairing for fp8 DoubleRow matmuls: 4 pairs with a constant
# address delta between the two shifted windows, plus tap 8 standalone
DW_PAIRS = ((0, 1), (3, 4), (6, 7), (2, 5))
W8_BRANCH = 4 * 256 + 128      # cols per branch in the fp8 weight block
W8_COLS = 2 * W8_BRANCH

N_CORES = 8
B_FULL = 32
B = B_FULL // N_CORES          # images per core
C = 256
H = W = 56
HW = H * W                     # 3136
PW = H + 2                     # 58 padded
NT = 7                         # pixel tiles per image
TS = HW // NT                  # 448 = 8 rows of 56
RPT = H // NT                  # 8 rows per tile
HEAD_DIM = 32
SCALER = HEAD_DIM ** -0.5
WIN = 7
HP = H // WIN                  # 8
POOL_N = HP * HP               # 64

OFFV = (1, 4)                  # tiles whose dw-v runs on DVE, not PE

# f16 weight block column offsets
WCOL = {}
_off = 0
for _nm, _w in (("wqkv0", 384), ("wqkv1", 384), ("dwdiag", 1152),
                ("wact1", 128), ("wact2", 128), ("wgq0", 128),
                ("wgq1", 128), ("wgkv0", 256), ("wgkv1", 256),
                ("wproj0", 256), ("wproj1", 256), ("denmask0", 128),
                ("denmask1", 128)):
    WCOL[_nm] = (_off, _off + _w)
    _off += _w
W16_COLS = _off                # 6016
W32_COLS = 14                  # dwb q,k,v | bact1 | bact2 | dwv taps 0..8


def _body(ctx, tc, d, n_img=B):
    nc = tc.nc

    # ---------------- persistent weights (2 consolidated DMAs) ----------
    wpool = ctx.enter_context(tc.tile_pool(name="wpool", bufs=1))

    warm_src = wpool.tile([128, 64], F16, tag="warm_src", name="warm_src")
    nc.vector.memset(warm_src, 0.0)

    wf16 = wpool.tile([128, W16_COLS], F16, tag="wf16", name="wf16")
    nc.sync.dma_start(out=wf16, in_=d["wf16"])
    wf32 = wpool.tile([128, W32_COLS], F32, tag="wf32", name="wf32")
    nc.sync.dma_start(out=wf32, in_=d["wf32"])
    wf8 = wpool.tile([128, W8_COLS], F8, tag="wf8", name="wf8")
    nc.sync.dma_start(out=wf8, in_=d["wf8"])

    def wv(name):
        a, b_ = WCOL[name]
        return wf16[:, a:b_]

    wqkv = [wv("wqkv0"), wv("wqkv1")]
    dwdiag = wv("dwdiag")
    wact1 = wv("wact1")
    wact2 = wv("wact2")
    wgq = [wv("wgq0"), wv("wgq1")]
    wgkv = [wv("wgkv0"), wv("wgkv1")]
    wproj = [wv("wproj0"), wv("wproj1")]
    denmask = [wv("denmask0"), wv("denmask1")]
    bias_q = wf32[:, 0:1]
    bias_k = wf32[:, 1:2]
    bias_v = wf32[:, 2:3]
    bact1 = wf32[:, 3:4]
    bact2 = wf32[:, 4:5]

    def wv_tap(tap):
        return wf32[:, 5 + tap:6 + tap]

    def dw_lhsT(cc, tap):
        return dwdiag[:, tap * 128:(tap + 1) * 128]    # v branch only

    # padded z buffers, x2 for image parity (borders stay zero; interiors
    # rewritten per image).  q/k are fp8 (read only by the DoubleRow dw
    # matmuls); v stays fp16.
    zbufs = []
    for par in range(2):
        zs = [wpool.tile([128, PW * PW], F8 if j < 2 else F16,
                         tag=f"z{j}_{par}", name=f"z{j}_{par}")
              for j in range(3)]
        for z in zs:
            zg = z.rearrange("p (r c) -> p r c", c=PW)
            nc.vector.memset(zg[:, 0, :], 0.0)          # top border row
            nc.vector.memset(zg[:, PW - 1, :], 0.0)     # bottom border row
            nc.vector.memset(zg[:, :, 0], 0.0)          # left border col
            nc.vector.memset(zg[:, :, PW - 1], 0.0)     # right border col
        zbufs.append(zs)

    # block-diagonal gk (2 heads per matmul at K=128) and zero-padded AV
    # lhsT blocks, x2 parity; zero regions never rewritten -> memset once
    gk2 = []
    av_lhs = []
    for par in range(2):
        g = [wpool.tile([128, 128], F16, tag=f"gk2_{p}_{par}",
                        name=f"gk2_{p}_{par}") for p in range(2)]
        a = [wpool.tile([128, 128], F16, tag=f"av_{p}_{par}",
                        name=f"av_{p}_{par}") for p in range(2)]
        for tbuf in (*g, *a):
            nc.vector.memset(tbuf, 0.0)
        gk2.append(g)
        av_lhs.append(a)

    # ---------------- pools ----------------
    ps = ctx.enter_context(tc.tile_pool(name="ps", bufs=4, space="PSUM"))
    xpool = ctx.enter_context(tc.tile_pool(name="xpool", bufs=4))
    big = ctx.enter_context(tc.tile_pool(name="big", bufs=1))
    sm = ctx.enter_context(tc.tile_pool(name="sm", bufs=3))
    tiny = ctx.enter_context(tc.tile_pool(name="tiny", bufs=2))

    gq_sb2 = [big.tile([128, HW], F16, tag=f"gq_sb{i}", name=f"gq_sb{i}")
              for i in range(2)]
    exp_sb = [big.tile([128, HW], F16, tag=f"exp{p}", name=f"exp{p}")
              for p in range(2)]
    rec_rep = big.tile([128, HW], F32, tag="rec_rep")
    cat_hi2 = [big.tile([128, HW], F16, tag=f"cat_hi{i}", name=f"cat_hi{i}")
               for i in range(2)]
    cat_lo2 = [big.tile([128, HW], F16, tag=f"cat_lo{i}", name=f"cat_lo{i}")
               for i in range(2)]

    zgrid = {id(z): z.rearrange("p (r c) -> p r c", c=PW)
             for zs in zbufs for z in zs}

    def zwin(z, t, dy, dx):
        r0 = RPT * t + dy
        return zgrid[id(z)][:, r0:r0 + RPT, dx:dx + W]

    def zint(z, t):
        r0 = RPT * t + 1
        return zgrid[id(z)][:, r0:r0 + RPT, 1:1 + W]

    # PE warmup: dummy matmuls keep the HAM clock-gate busy while the
    # weight/x DMAs land, so real matmuls start at 2.4 GHz
    for wi in range(48):
        pw = ps.tile([64, 64], F32, tag="py", name="pwarm")
        nc.tensor.matmul(pw[:], warm_src[:], warm_src[:],
                         start=True, stop=True)

    # ---------------- stage helpers ----------------
    def load_x(b):
        x_sb = [xpool.tile([128, HW], F16, tag=f"x{cc}", name=f"x{cc}")
                for cc in range(2)]
        for cc in range(2):
            nc.sync.dma_start(out=x_sb[cc], in_=d["x"][b, cc])
        return x_sb

    def qkv_tile(b, t, x_sb):
        z_q, z_k, z_v = zbufs[b % 2]
        for j, (z, eng) in enumerate(
                ((z_q, "act"), (z_k, "act"), (z_v, "dve"))):
            pq = ps.tile([128, TS], F32, tag="py", name="pq")
            for cc in range(2):
                nc.tensor.matmul(
                    pq[:], wqkv[cc][:, j * 128:(j + 1) * 128],
                    x_sb[cc][:, t * TS:(t + 1) * TS],
                    start=(cc == 0), stop=(cc == 1))
            if eng == "act":
                nc.scalar.copy(out=zint(z, t), in_=pq[:])
            else:
                nc.vector.tensor_copy(out=zint(z, t), in_=pq[:])

    def gq_tile(b, t, x_sb):
        pg = ps.tile([128, TS], F32, tag="py", name="pg")
        for cc in range(2):
            nc.tensor.matmul(pg[:], wgq[cc][:],
                             x_sb[cc][:, t * TS:(t + 1) * TS],
                             start=(cc == 0), stop=(cc == 1))
        nc.vector.tensor_copy(out=gq_sb2[b % 2][:, t * TS:(t + 1) * TS],
                              in_=pg[:])

    def pool_reduce(x_sb):
        """7x7 window sums; issued early so the results have slack."""
        pooled = []
        for cc in range(2):
            pr1 = sm.tile([128, H * HP], F32, tag="pr1", name="pr1")
            nc.vector.tensor_reduce(
                out=pr1.rearrange("p (y g) -> p y g", g=HP),
                in_=x_sb[cc].rearrange("p (y g x) -> p y g x", y=H, g=HP),
                axis=mybir.AxisListType.X, op=OP.add)
            po = tiny.tile([128, POOL_N], F16, tag="po", name="po")
            with nc.allow_low_precision(reason="pool sums fit fp16"):
                nc.vector.tensor_reduce(
                    out=po.rearrange("p (a b) -> p a b", a=HP),
                    in_=pr1.rearrange("p (hp dy wp) -> p hp wp dy",
                                      hp=HP, dy=WIN),
                    axis=mybir.AxisListType.X, op=OP.add)
            pooled.append(po)
        return pooled

    def pool_finish(b, pooled):
        """global-kv matmuls + lhsT packing for image b's attention."""
        par = b % 2
        pgk = ps.tile([128, POOL_N], F32, tag="py", name="pgk")
        for cc in range(2):
            nc.tensor.matmul(pgk[:], wgkv[cc][:, 0:128], pooled[cc][:],
                             start=(cc == 0), stop=(cc == 1))
        for p in range(2):
            for hl in range(2):
                h = 2 * p + hl
                nc.scalar.copy(
                    out=gk2[par][p][32 * h:32 * h + 32,
                                    64 * hl:64 * hl + 64],
                    in_=pgk[32 * h:32 * h + 32, :])
        pgv = ps.tile([POOL_N, 128], F32, tag="py", name="pgv")
        for cc in range(2):
            nc.tensor.matmul(pgv[:], pooled[cc][:], wgkv[cc][:, 128:256],
                             start=(cc == 0), stop=(cc == 1))
        gvT = tiny.tile([POOL_N, 128], F16, tag="gvT", name="gvT")
        nc.scalar.copy(out=gvT[:], in_=pgv[:])
        av0, av1 = av_lhs[par]
        nc.vector.tensor_copy(out=av0[0:64, 0:32], in_=gvT[:, 0:32])
        nc.sync.dma_start(out=av0[64:128, 32:64], in_=gvT[:, 32:64])
        nc.vector.tensor_copy(out=av1[0:64, 64:96], in_=gvT[:, 64:96])
        nc.sync.dma_start(out=av1[64:128, 96:128], in_=gvT[:, 96:128])

    def dw_mm(z, cc, t, psname):
        p = ps.tile([128, TS], F32, tag="px", name=psname)
        for tap in range(9):
            dy, dx = divmod(tap, 3)
            nc.tensor.matmul(p[:], dw_lhsT(cc, tap), zwin(z, t, dy, dx),
                             start=(tap == 0), stop=(tap == 8))
        return p

    def dw_mm8(z8, br, t, psname):
        """dw conv via 4 fp8 DoubleRow pair-matmuls + 1 plain fp8 matmul."""
        p = ps.tile([128, TS], F32, tag="px", name=psname)
        zg = zgrid[id(z8)]
        for pr, (tapA, tapB) in enumerate(DW_PAIRS):
            dyA, dxA = divmod(tapA, 3)
            dyB, dxB = divmod(tapB, 3)
            delta = (dyB - dyA) * PW + (dxB - dxA)
            w = zg[:, RPT * t + dyA:RPT * t + dyA + RPT, dxA:dxA + W]
            pa = list(w.ap)
            rhs = bass.AP(w.tensor, w.offset,
                          [pa[0], [delta, 2], pa[1], pa[2]])
            lhsT = wf8[:, br * W8_BRANCH + pr * 256:
                       br * W8_BRANCH + (pr + 1) * 256]
            nc.tensor.matmul(p[:], lhsT.rearrange("p (i m) -> p i m", i=2),
                             rhs, start=(pr == 0), stop=False,
                             perf_mode=mybir.MatmulPerfMode.DoubleRow)
        nc.tensor.matmul(p[:],
                         wf8[:, br * W8_BRANCH + 1024:br * W8_BRANCH + 1152],
                         zwin(z8, t, 2, 2), start=False, stop=True)
        return p

    def dwv_vector(z_v, t):
        """dw-v for one tile as 9 shifted multiply-adds on DVE; returns
        the accumulated (dwv + bias_v) tile in fp16."""
        acc = sm.tile([128, TS], F16, tag="accv", name="accv")
        with nc.allow_low_precision(reason="dwv fits fp16"):
            nc.vector.tensor_scalar(
                out=acc[:], in0=zwin(z_v, t, 0, 0), scalar1=wv_tap(0),
                scalar2=bias_v, op0=OP.mult, op1=OP.add)
            for tap in range(1, 9):
                dy, dx = divmod(tap, 3)
                nacc = sm.tile([128, TS], F16, tag="accv", name="accv")
                nc.vector.scalar_tensor_tensor(
                    out=nacc[:], in0=zwin(z_v, t, dy, dx),
                    scalar=wv_tap(tap), in1=acc[:],
                    op0=OP.mult, op1=OP.add)
                acc = nacc
        return acc

    # ---------------- pipelined loops ----------------
    def loop_a(b):
        """dwconv + gating chain, 2-tile skew."""
        z_q, z_k, z_v = zbufs[b % 2]
        cat_hi = cat_hi2[b % 2]
        qk_t = {}
        ta = {}
        hs = {}
        for i in range(NT + 2):
            if i < NT:
                t = i
                pdq = dw_mm8(z_q, 0, t, "pdq")
                q_t = sm.tile([128, TS], F16, tag="q_t", name="q_t")
                nc.scalar.activation(out=q_t[:], in_=pdq[:],
                                     func=AF.Identity, bias=bias_q)
                pdk = dw_mm8(z_k, 1, t, "pdk")
                qk = sm.tile([128, TS], F16, tag="qk_t", name="qk_t")
                with nc.allow_low_precision(reason="qk product fits fp16"):
                    nc.vector.scalar_tensor_tensor(
                        out=qk[:], in0=pdk[:], scalar=bias_k, in1=q_t[:],
                        op0=OP.add, op1=OP.mult)
                qk_t[t] = qk
            if 1 <= i <= NT:
                t = i - 1
                pa1 = ps.tile([128, TS], F32, tag="py", name="pa1")
                nc.tensor.matmul(pa1[:], wact1[:], qk_t[t][:],
                                 start=True, stop=True)
                t_a = sm.tile([128, TS], F16, tag="t_a", name="t_a")
                nc.scalar.activation(out=t_a[:], in_=pa1[:],
                                     func=AF.Identity, bias=bact1)
                u_t = sm.tile([128, TS], F16, tag="u_t", name="u_t")
                nc.vector.tensor_scalar(out=u_t[:], in0=t_a[:], scalar1=3.0,
                                        scalar2=0.0, op0=OP.add, op1=OP.max)
                h_t = sm.tile([128, TS], F16, tag="hs_t", name="hs_t")
                with nc.allow_low_precision(reason="hardswish fits fp16"):
                    nc.vector.scalar_tensor_tensor(
                        out=h_t[:], in0=u_t[:], scalar=6.0, in1=t_a[:],
                        op0=OP.min, op1=OP.mult)
                ta[t] = t_a
                hs[t] = h_t
            if 2 <= i:
                t = i - 2
                sl = slice(t * TS, (t + 1) * TS)
                pa2 = ps.tile([128, TS], F32, tag="py", name="pa2")
                nc.tensor.matmul(pa2[:], wact2[:], hs[t][:],
                                 start=True, stop=True)
                g_t = sm.tile([128, TS], F16, tag="g_t", name="g_t")
                nc.scalar.activation(out=g_t[:], in_=pa2[:], func=AF.Tanh,
                                     bias=bact2)
                if t in OFFV:
                    acc = dwv_vector(z_v, t)
                    with nc.allow_low_precision(reason="gated out fp16"):
                        nc.vector.scalar_tensor_tensor(
                            out=cat_hi[:, sl], in0=acc[:], scalar=1.0,
                            in1=g_t[:], op0=OP.mult, op1=OP.mult)
                else:
                    pdv = dw_mm(z_v, 2, t, "pdv")
                    v_t = sm.tile([128, TS], F16, tag="v_t", name="v_t")
                    nc.scalar.activation(out=v_t[:], in_=pdv[:],
                                         func=AF.Identity, bias=bias_v)
                    with nc.allow_low_precision(reason="gated out fp16"):
                        nc.vector.scalar_tensor_tensor(
                            out=cat_hi[:, sl], in0=v_t[:], scalar=1.0,
                            in1=g_t[:], op0=OP.mult, op1=OP.mult)

    def loop_b(b, x_next, pooled_next):
        """attention + projection for image b, interleaved with the
        qkv/gq front-end of image b+1."""
        par = b % 2
        cat_hi = cat_hi2[par]
        cat_lo = cat_lo2[par]
        gq_sb = gq_sb2[par]
        av0, av1 = av_lhs[par]
        for i in range(NT + 3):
            if i < NT:
                t = i
                sl = slice(t * TS, (t + 1) * TS)
                if x_next is not None:
                    qkv_tile(b + 1, t, x_next)
                    gq_tile(b + 1, t, x_next)
                for p in range(2):
                    pat = ps.tile([128, TS], F32, tag="px", name="pat")
                    nc.tensor.matmul(pat[:], gk2[par][p][:], gq_sb[:, sl],
                                     start=True, stop=True)
                    nc.scalar.activation(out=exp_sb[p][:, sl], in_=pat[:],
                                         func=AF.Exp, scale=float(SCALER))
            if 1 <= i <= NT:
                t = i - 1
                sl = slice(t * TS, (t + 1) * TS)
                pden = ps.tile([128, TS], F32, tag="px", name="pden")
                for p in range(2):
                    nc.tensor.matmul(pden[:], denmask[p][:],
                                     exp_sb[p][:, sl],
                                     start=(p == 0), stop=(p == 1))
                nc.vector.reciprocal_approx_fast(out=rec_rep[:, sl],
                                                 in_=pden[:])
            if 2 <= i <= NT + 1:
                t = i - 2
                sl = slice(t * TS, (t + 1) * TS)
                pav = ps.tile([128, TS], F32, tag="px", name="pav")
                nc.tensor.matmul(pav[:], av0[:], exp_sb[0][:, sl],
                                 start=True, stop=False)
                nc.tensor.matmul(pav[:], av1[:], exp_sb[1][:, sl],
                                 start=False, stop=True)
                with nc.allow_low_precision(reason="attn out fits fp16"):
                    nc.vector.scalar_tensor_tensor(
                        out=cat_lo[:, sl], in0=pav[:], scalar=1.0,
                        in1=rec_rep[:, sl], op0=OP.mult, op1=OP.mult)
            if 3 <= i:
                t = i - 3
                sl = slice(t * TS, (t + 1) * TS)
                for m in range(2):
                    pp = ps.tile([128, TS], F32, tag="py", name="pp")
                    nc.tensor.matmul(pp[:],
                                     wproj[0][:, m * 128:(m + 1) * 128],
                                     cat_hi[:, sl], start=True, stop=False)
                    nc.tensor.matmul(pp[:],
                                     wproj[1][:, m * 128:(m + 1) * 128],
                                     cat_lo[:, sl], start=False, stop=True)
                    o_t = sm.tile([128, TS], F16, tag=f"o_t{m}",
                                  name=f"o_t{m}")
                    if m == 0:
                        nc.scalar.copy(out=o_t[:], in_=pp[:])
                    else:
                        nc.vector.tensor_copy(out=o_t[:], in_=pp[:])
                    nc.sync.dma_start(out=d["out"][b, m, :, sl], in_=o_t)
        if pooled_next is not None:
            pool_finish(b + 1, pooled_next)

    # ---------------- program ----------------
    x_cur = load_x(0)
    pooled = pool_reduce(x_cur)
    for t in range(NT):
        qkv_tile(0, t, x_cur)
        gq_tile(0, t, x_cur)
    pool_finish(0, pooled)

    for b in range(n_img):
        if b + 1 < n_img:
            x_next = load_x(b + 1)
            pooled_next = pool_reduce(x_next)
        else:
            x_next = pooled_next = None
        loop_a(b)
        loop_b(b, x_next, pooled_next)


def _build(n_img=B):
    nc = bacc.Bacc("TRN2", target_bir_lowering=False, debug=False,
                   num_devices=N_CORES)
    dt = nc.dram_tensor
    d = {
        "x": dt("x", [B, 2, 128, HW], F16, kind="ExternalInput").ap(),
        "wf16": dt("wf16", [128, W16_COLS], F16, kind="ExternalInput").ap(),
        "wf32": dt("wf32", [128, W32_COLS], F32, kind="ExternalInput").ap(),
        "wf8": dt("wf8", [128, W8_COLS], F8, kind="ExternalInput").ap(),
        "out": dt("out", [B, 2, 128, HW], F16, kind="ExternalOutput").ap(),
    }
    with tile.TileContext(nc) as tc, ExitStack() as ctx:
        _body(ctx, tc, d, n_img=n_img)
    nc.compile()
    return nc


_NC = None


def _prep_weights(qkv_w, dw_w, dw_b, act1_w, act1_b, act2_w, act2_b,
                  gq_w, gkv_w, proj_w):
    f32 = np.float32
    f16 = np.float16
    sc = np.float32(HEAD_DIM ** -0.5)

    wqkv = qkv_w.T.reshape(2, 128, 384).astype(f16)
    taps = dw_w.reshape(384, 9)            # [c, tap]
    idx = np.arange(128)
    # f16 diag blocks for the v branch only
    dwd = np.zeros((9, 128, 128), dtype=f16)
    for tp in range(9):
        dwd[tp, idx, idx] = taps[256:384, tp]
    dwdiag = dwd.transpose(1, 0, 2).reshape(128, 9 * 128)
    # fp8 DoubleRow pair blocks for q and k branches
    f8 = np.dtype(np.float32)  # placeholder; real cast below
    import ml_dtypes
    e4 = ml_dtypes.float8_e4m3
    wf8 = np.zeros((128, W8_COLS), dtype=e4)
    for br in range(2):
        tb = taps[128 * br:128 * (br + 1)]
        for pr, (ta_, tb_) in enumerate(DW_PAIRS):
            blk = np.zeros((128, 2, 128), np.float32)
            blk[idx, 0, idx] = tb[:, ta_]
            blk[idx, 1, idx] = tb[:, tb_]
            wf8[:, br * W8_BRANCH + pr * 256:
                br * W8_BRANCH + (pr + 1) * 256] = (
                blk.reshape(128, 256).astype(e4))
        t8 = np.zeros((128, 128), np.float32)
        t8[idx, idx] = tb[:, 8]
        wf8[:, br * W8_BRANCH + 1024:br * W8_BRANCH + 1152] = t8.astype(e4)
    wact1 = (act1_w * sc).T.astype(f16)
    wact2 = (act2_w / 6.0).T.astype(f16)
    wgq = gq_w.T.reshape(2, 128, 128).astype(f16)
    wgkv = (gkv_w / 49.0).T.reshape(2, 128, 256).astype(f16)
    wproj = proj_w.T.reshape(2, 128, 256).astype(f16)
    dm = np.zeros((2, 128, 128), dtype=f16)
    for p in range(2):
        for hl in range(2):
            head = 2 * p + hl
            dm[p, 64 * hl:64 * hl + 64, 32 * head:32 * head + 32] = 1.0

    blocks = {"wqkv0": wqkv[0], "wqkv1": wqkv[1], "dwdiag": dwdiag,
              "wact1": wact1, "wact2": wact2, "wgq0": wgq[0],
              "wgq1": wgq[1], "wgkv0": wgkv[0], "wgkv1": wgkv[1],
              "wproj0": wproj[0], "wproj1": wproj[1],
              "denmask0": dm[0], "denmask1": dm[1]}
    wf16 = np.zeros((128, W16_COLS), dtype=f16)
    for nm, (a, b_) in WCOL.items():
        wf16[:, a:b_] = blocks[nm]

    wf32 = np.zeros((128, W32_COLS), dtype=f32)
    wf32[:, 0:3] = dw_b.reshape(3, 128).T
    wf32[:, 3] = act1_b.astype(f32)
    wf32[:, 4] = act2_b.astype(f32)
    wf32[:, 5:14] = taps[256:384].astype(f32)   # dw-v taps for DVE path

    return {"wf16": np.ascontiguousarray(wf16),
            "wf32": np.ascontiguousarray(wf32),
            "wf8": np.ascontiguousarray(wf8)}


def kernel(**inputs):
    global _NC
    x = inputs["x"]
    w = _prep_weights(
        inputs["qkv_w"], inputs["dw_w"], inputs["dw_b"],
        inputs["act1_w"], inputs["act1_b"], inputs["act2_w"],
        inputs["act2_b"], inputs["gq_w"], inputs["gkv_w"], inputs["proj_w"])
    if _NC is None:
        _NC = _build()
    in_maps = []
    for core in range(N_CORES):
        m = dict(w)
        m["x"] = np.ascontiguousarray(
            x[core * B:(core + 1) * B].reshape(B, 2, 128, HW)
            .astype(np.float16))
        in_maps.append(m)
    res = run_bass_kernel_spmd(_NC, in_maps, core_ids=list(range(N_CORES)))
    out = np.concatenate([r["out"] for r in res.results], axis=0)
    return out.reshape(B_FULL, C, H, W).astype(np.float32)


# revision 34
# speedup vs baseline: 2.3307x; 1.0138x over previous
"""CloAttention Trainium2 Bass kernel.

Full inputs -> data-parallel over batch across 8 NeuronCores (4 images each)
-> full output.  All matmuls run on the PE in fp16 (1 cycle/row); the 3x3
depthwise conv runs as 9 diagonal-matmul accumulations into PSUM.

Schedule: software-pipelined per image.  Loop A runs the depthwise/gating
chain with a 2-tile skew so the PE never waits on the scalar/vector chain;
loop B runs attention + projection for image b interleaved with the qkv/gq
front-end of image b+1.  Pooling runs on the otherwise-idle GPSIMD engine;
a couple of dw-v tiles per image run as shifted multiply-adds on DVE to
shave PE work.  Weights arrive in two consolidated DMAs and dummy matmuls
warm the PE HAM clock-gate during the initial DMA wait.
"""

import numpy as np
from contextlib import ExitStack

import concourse.bacc as bacc
import concourse.bass as bass
import concourse.tile as tile
from concourse import mybir
from concourse.bass_utils import run_bass_kernel_spmd

F32 = mybir.dt.float32
F16 = mybir.dt.float16
F8 = mybir.dt.float8e4
AF = mybir.ActivationFunctionType
OP = mybir.AluOpType

# dw tap pairing for fp8 DoubleRow matmuls: 4 pairs with a constant
# address delta between the two shifted windows, plus tap 8 standalone
DW_PAIRS = ((0, 1), (3, 4), (6, 7), (2, 5))
W8_BRANCH = 4 * 256 + 128      # cols per branch in the fp8 weight block
W8_COLS = 2 * W8_BRANCH

N_CORES = 8
B_FULL = 32
B = B_FULL // N_CORES          # images per core
C = 256
H = W = 56
HW = H * W                     # 3136
PW = H + 2                     # 58 padded
NT = 7                         # pixel tiles per image
TS = HW // NT                  # 448 = 8 rows of 56
RPT = H // NT                  # 8 rows per tile
HEAD_DIM = 32
SCALER = HEAD_DIM ** -0.5
WIN = 7
HP = H // WIN                  # 8
POOL_N = HP * HP               # 64

OFFV = (1, 4)                  # tiles whose dw-v runs on DVE, not PE

# f16 weight block column offsets
WCOL = {}
_off = 0
for _nm, _w in (("wqkv0", 384), ("wqkv1", 384), ("dwdiag", 1152),
                ("wact1", 128), ("wact2", 128), ("wgq0", 128),
                ("wgq1", 128), ("wgkv0", 256), ("wgkv1", 256),
                ("wproj0", 256), ("wproj1", 256), ("denmask0", 128),
                ("denmask1", 128)):
    WCOL[_nm] = (_off, _off + _w)
    _off += _w
W16_COLS = _off                # 6016
W32_COLS = 14                  # dwb q,k,v | bact1 | bact2 | dwv taps 0..8


def _body(ctx, tc, d, n_img=B):
    nc = tc.nc

    # ---------------- persistent weights (2 consolidated DMAs) ----------
    wpool = ctx.enter_context(tc.tile_pool(name="wpool", bufs=1))

    warm_src = wpool.tile([128, 64], F16, tag="warm_src", name="warm_src")
    nc.vector.memset(warm_src, 0.0)

    wf16 = wpool.tile([128, W16_COLS], F16, tag="wf16", name="wf16")
    nc.sync.dma_start(out=wf16, in_=d["wf16"])
    wf32 = wpool.tile([128, W32_COLS], F32, tag="wf32", name="wf32")
    nc.sync.dma_start(out=wf32, in_=d["wf32"])
    wf8 = wpool.tile([128, W8_COLS], F8, tag="wf8", name="wf8")
    nc.sync.dma_start(out=wf8, in_=d["wf8"])

    def wv(name):
        a, b_ = WCOL[name]
        return wf16[:, a:b_]

    wqkv = [wv("wqkv0"), wv("wqkv1")]
    dwdiag = wv("dwdiag")
    wact1 = wv("wact1")
    wact2 = wv("wact2")
    wgq = [wv("wgq0"), wv("wgq1")]
    wgkv = [wv("wgkv0"), wv("wgkv1")]
    wproj = [wv("wproj0"), wv("wproj1")]
    denmask = [wv("denmask0"), wv("denmask1")]
    bias_q = wf32[:, 0:1]
    bias_k = wf32[:, 1:2]
    bias_v = wf32[:, 2:3]
    bact1 = wf32[:, 3:4]
    bact2 = wf32[:, 4:5]

    def wv_tap(tap):
        return wf32[:, 5 + tap:6 + tap]

    def dw_lhsT(cc, tap):
        return dwdiag[:, tap * 128:(tap + 1) * 128]    # v branch only

    # padded z buffers, x2 for image parity (borders stay zero; interiors
    # rewritten per image).  q/k are fp8 (read only by the DoubleRow dw
    # matmuls); v stays fp16.
    zbufs = []
    for par in range(2):
        zs = [wpool.tile([128, PW * PW], F8 if j < 2 else F16,
                         tag=f"z{j}_{par}", name=f"z{j}_{par}")
              for j in range(3)]
        for z in zs:
            zg = z.rearrange("p (r c) -> p r c", c=PW)
            nc.vector.memset(zg[:, 0, :], 0.0)          # top border row
            nc.vector.memset(zg[:, PW - 1, :], 0.0)     # bottom border row
            nc.vector.memset(zg[:, :, 0], 0.0)          # left border col
            nc.vector.memset(zg[:, :, PW - 1], 0.0)     # right border col
        zbufs.append(zs)

    # block-diagonal gk (2 heads per matmul at K=128) and zero-padded AV
    # lhsT blocks, x2 parity; zero regions never rewritten -> memset once
    gk2 = []
    av_lhs = []
    for par in range(2):
        g = [wpool.tile([128, 128], F16, tag=f"gk2_{p}_{par}",
                        name=f"gk2_{p}_{par}") for p in range(2)]
        a = [wpool.tile([128, 128], F16, tag=f"av_{p}_{par}",
                        name=f"av_{p}_{par}") for p in range(2)]
        for tbuf in (*g, *a):
            nc.vector.memset(tbuf, 0.0)
        gk2.append(g)
        av_lhs.append(a)

    # ---------------- pools ----------------
    ps = ctx.enter_context(tc.tile_pool(name="ps", bufs=4, space="PSUM"))
    xpool = ctx.enter_context(tc.tile_pool(name="xpool", bufs=4))
    big = ctx.enter_context(tc.tile_pool(name="big", bufs=1))
    sm = ctx.enter_context(tc.tile_pool(name="sm", bufs=3))
    tiny = ctx.enter_context(tc.tile_pool(name="tiny", bufs=2))

    gq_sb2 = [big.tile([128, HW], F16, tag=f"gq_sb{i}", name=f"gq_sb{i}")
              for i in range(2)]
    exp_sb = [big.tile([128, HW], F16, tag=f"exp{p}", name=f"exp{p}")
              for p in range(2)]
    rec_rep = big.tile([128, HW], F32, tag="rec_rep")
    cat_hi2 = [big.tile([128, HW], F16, tag=f"cat_hi{i}", name=f"cat_hi{i}")
               for i in range(2)]
    cat_lo2 = [big.tile([128, HW], F16, tag=f"cat_lo{i}", name=f"cat_lo{i}")
               for i in range(2)]

    zgrid = {id(z): z.rearrange("p (r c) -> p r c", c=PW)
             for zs in zbufs for z in zs}

    def zwin(z, t, dy, dx):
        r0 = RPT * t + dy
        return zgrid[id(z)][:, r0:r0 + RPT, dx:dx + W]

    def zint(z, t):
        r0 = RPT * t + 1
        return zgrid[id(z)][:, r0:r0 + RPT, 1:1 + W]

    # PE warmup: dummy matmuls keep the HAM clock-gate busy while the
    # weight/x DMAs land, so real matmuls start at 2.4 GHz
    for wi in range(48):
        pw = ps.tile([64, 64], F32, tag="py", name="pwarm")
        nc.tensor.matmul(pw[:], warm_src[:], warm_src[:],
                         start=True, stop=True)

    # ---------------- stage helpers ----------------
    def load_x(b):
        x_sb = [xpool.tile([128, HW], F16, tag=f"x{cc}", name=f"x{cc}")
                for cc in range(2)]
        for cc in range(2):
            nc.sync.dma_start(out=x_sb[cc], in_=d["x"][b, cc])
        return x_sb

    def qkv_tile(b, t, x_sb):
        z_q, z_k, z_v = zbufs[b % 2]
        for j, (z, eng) in enumerate(
                ((z_q, "act"), (z_k, "act"), (z_v, "dve"))):
            pq = ps.tile([128, TS], F32, tag="py", name="pq")
            for cc in range(2):
                nc.tensor.matmul(
                    pq[:], wqkv[cc][:, j * 128:(j + 1) * 128],
                    x_sb[cc][:, t * TS:(t + 1) * TS],
                    start=(cc == 0), stop=(cc == 1))
            if eng == "act":
                nc.scalar.copy(out=zint(z, t), in_=pq[:])
            else:
                nc.vector.tensor_copy(out=zint(z, t), in_=pq[:])

    def gq_tile(b, t, x_sb):
        pg = ps.tile([128, TS], F32, tag="py", name="pg")
        for cc in range(2):
            nc.tensor.matmul(pg[:], wgq[cc][:],
                             x_sb[cc][:, t * TS:(t + 1) * TS],
                             start=(cc == 0), stop=(cc == 1))
        nc.vector.tensor_copy(out=gq_sb2[b % 2][:, t * TS:(t + 1) * TS],
                              in_=pg[:])

    def pool_reduce(x_sb):
        """7x7 window sums; issued early so the results have slack."""
        pooled = []
        for cc in range(2):
            pr1 = sm.tile([128, H * HP], F32, tag="pr1", name="pr1")
            nc.vector.tensor_reduce(
                out=pr1.rearrange("p (y g) -> p y g", g=HP),
                in_=x_sb[cc].rearrange("p (y g x) -> p y g x", y=H, g=HP),
                axis=mybir.AxisListType.X, op=OP.add)
            po = tiny.tile([128, POOL_N], F16, tag="po", name="po")
            with nc.allow_low_precision(reason="pool sums fit fp16"):
                nc.vector.tensor_reduce(
                    out=po.rearrange("p (a b) -> p a b", a=HP),
                    in_=pr1.rearrange("p (hp dy wp) -> p hp wp dy",
                                      hp=HP, dy=WIN),
                    axis=mybir.AxisListType.X, op=OP.add)
            pooled.append(po)
        return pooled

    def pool_finish(b, pooled):
        """global-kv matmuls + lhsT packing for image b's attention."""
        par = b % 2
        pgk = ps.tile([128, POOL_N], F32, tag="py", name="pgk")
        for cc in range(2):
            nc.tensor.matmul(pgk[:], wgkv[cc][:, 0:128], pooled[cc][:],
                             start=(cc == 0), stop=(cc == 1))
        for p in range(2):
            for hl in range(2):
                h = 2 * p + hl
                nc.scalar.copy(
                    out=gk2[par][p][32 * h:32 * h + 32,
                                    64 * hl:64 * hl + 64],
                    in_=pgk[32 * h:32 * h + 32, :])
        pgv = ps.tile([POOL_N, 128], F32, tag="py", name="pgv")
        for cc in range(2):
            nc.tensor.matmul(pgv[:], pooled[cc][:], wgkv[cc][:, 128:256],
                             start=(cc == 0), stop=(cc == 1))
        gvT = tiny.tile([POOL_N, 128], F16, tag="gvT", name="gvT")
        nc.scalar.copy(out=gvT[:], in_=pgv[:])
        av0, av1 = av_lhs[par]
        nc.vector.tensor_copy(out=av0[0:64, 0:32], in_=gvT[:, 0:32])
        nc.sync.dma_start(out=av0[64:128, 32:64], in_=gvT[:, 32:64])
        nc.vector.tensor_copy(out=av1[0:64, 64:96], in_=gvT[:, 64:96])
        nc.sync.dma_start(out=av1[64:128, 96:128], in_=gvT[:, 96:128])

    def dw_mm(z, cc, t, psname):
        p = ps.tile([128, TS], F32, tag="px", name=psname)
        for tap in range(9):
            dy, dx = divmod(tap, 3)
            nc.tensor.matmul(p[:], dw_lhsT(cc, tap), zwin(z, t, dy, dx),
                             start=(tap == 0), stop=(tap == 8))
        return p

    def dw_mm8(z8, br, t, psname):
        """dw conv via 4 fp8 DoubleRow pair-matmuls + 1 plain fp8 matmul."""
        p = ps.tile([128, TS], F32, tag="px", name=psname)
        zg = zgrid[id(z8)]
        for pr, (tapA, tapB) in enumerate(DW_PAIRS):
            dyA, dxA = divmod(tapA, 3)
            dyB, dxB = divmod(tapB, 3)
            delta = (dyB - dyA) * PW + (dxB - dxA)
            w = zg[:, RPT * t + dyA:RPT * t + dyA + RPT, dxA:dxA + W]
            pa = list(w.ap)
            rhs = bass.AP(w.tensor, w.offset,
                          [pa[0], [delta, 2], pa[1], pa[2]])
            lhsT = wf8[:, br * W8_BRANCH + pr * 256:
                       br * W8_BRANCH + (pr + 1) * 256]
            nc.tensor.matmul(p[:], lhsT.rearrange("p (i m) -> p i m", i=2),
                             rhs, start=(pr == 0), stop=False,
                             perf_mode=mybir.MatmulPerfMode.DoubleRow)
        nc.tensor.matmul(p[:],
                         wf8[:, br * W8_BRANCH + 1024:br * W8_BRANCH + 1152],
                         zwin(z8, t, 2, 2), start=False, stop=True)
        return p

    def dwv_vector(z_v, t):
        """dw-v for one tile as 9 shifted multiply-adds on DVE; returns
        the accumulated (dwv + bias_v) tile in fp16."""
        acc = sm.tile([128, TS], F16, tag="accv", name="accv")
        with nc.allow_low_precision(reason="dwv fits fp16"):
            nc.vector.tensor_scalar(
                out=acc[:], in0=zwin(z_v, t, 0, 0), scalar1=wv_tap(0),
                scalar2=bias_v, op0=OP.mult, op1=OP.add)
            for tap in range(1, 9):
                dy, dx = divmod(tap, 3)
                nacc = sm.tile([128, TS], F16, tag="accv", name="accv")
                nc.vector.scalar_tensor_tensor(
                    out=nacc[:], in0=zwin(z_v, t, dy, dx),
                    scalar=wv_tap(tap), in1=acc[:],
                    op0=OP.mult, op1=OP.add)
                acc = nacc
        return acc

    # ---------------- pipelined loops ----------------
    def loop_a(b):
        """dwconv + gating chain, 2-tile skew."""
        z_q, z_k, z_v = zbufs[b % 2]
        cat_hi = cat_hi2[b % 2]
        qk_t = {}
        ta = {}
        hs = {}
        for i in range(NT + 2):
            if i < NT:
                t = i
                pdq = dw_mm8(z_q, 0, t, "pdq")
                q_t = sm.tile([128, TS], F16, tag="q_t", name="q_t")
                nc.scalar.activation(out=q_t[:], in_=pdq[:],
                                     func=AF.Identity, bias=bias_q)
                pdk = dw_mm8(z_k, 1, t, "pdk")
                qk = sm.tile([128, TS], F16, tag="qk_t", name="qk_t")
                with nc.allow_low_precision(reason="qk product fits fp16"):
                    nc.vector.scalar_tensor_tensor(
                        out=qk[:], in0=pdk[:], scalar=bias_k, in1=q_t[:],
                        op0=OP.add, op1=OP.mult)
                qk_t[t] = qk
            if 1 <= i <= NT:
                t = i - 1
                pa1 = ps.tile([128, TS], F32, tag="py", name="pa1")
                nc.tensor.matmul(pa1[:], wact1[:], qk_t[t][:],
                                 start=True, stop=True)
                t_a = sm.tile([128, TS], F16, tag="t_a", name="t_a")
                nc.scalar.activation(out=t_a[:], in_=pa1[:],
                                     func=AF.Identity, bias=bact1)
                u_t = sm.tile([128, TS], F16, tag="u_t", name="u_t")
                nc.vector.tensor_scalar(out=u_t[:], in0=t_a[:], scalar1=3.0,
                                        scalar2=0.0, op0=OP.add, op1=OP.max)
                h_t = sm.tile([128, TS], F16, tag="hs_t", name="hs_t")
                with nc.allow_low_precision(reason="hardswish fits fp16"):
                    nc.vector.scalar_tensor_tensor(
                        out=h_t[:], in0=u_t[:], scalar=6.0, in1=t_a[:],
                        op0=OP.min, op1=OP.mult)
                ta[t] = t_a
                hs[t] = h_t
            if 2 <= i:
                t = i - 2
                sl = slice(t * TS, (t + 1) * TS)
                pa2 = ps.tile([128, TS], F32, tag="py", name="pa2")
                nc.tensor.matmul(pa2[:], wact2[:], hs[t][:],
                                 start=True, stop=True)
                g_t = sm.tile([128, TS], F16, tag="g_t", name="g_t")
                nc.scalar.activation(out=g_t[:], in_=pa2[:], func=AF.Tanh,
                                     bias=bact2)
                if t in OFFV:
                    acc = dwv_vector(z_v, t)
                    with nc.allow_low_precision(reason="gated out fp16"):
                        nc.vector.scalar_tensor_tensor(
                            out=cat_hi[:, sl], in0=acc[:], scalar=1.0,
                            in1=g_t[:], op0=OP.mult, op1=OP.mult)
                else:
                    pdv = dw_mm(z_v, 2, t, "pdv")
                    v_t = sm.tile([128, TS], F16, tag="v_t", name="v_t")
                    nc.scalar.activation(out=v_t[:], in_=pdv[:],
                                         func=AF.Identity, bias=bias_v)
                    with nc.allow_low_precision(reason="gated out fp16"):
                        nc.vector.scalar_tensor_tensor(
                            out=cat_hi[:, sl], in0=v_t[:], scalar=1.0,
                            in1=g_t[:], op0=OP.mult, op1=OP.mult)

    def loop_b(b, x_next, pooled_next):
        """attention + projection for image b, interleaved with the
        qkv/gq front-end of image b+1."""
        par = b % 2
        cat_hi = cat_hi2[par]
        cat_lo = cat_lo2[par]
        gq_sb = gq_sb2[par]
        av0, av1 = av_lhs[par]
        for i in range(NT + 3):
            if i < NT:
                t = i
                sl = slice(t * TS, (t + 1) * TS)
                if x_next is not None:
                    qkv_tile(b + 1, t, x_next)
                    gq_tile(b + 1, t, x_next)
                for p in range(2):
                    pat = ps.tile([128, TS], F32, tag="px", name="pat")
                    nc.tensor.matmul(pat[:], gk2[par][p][:], gq_sb[:, sl],
                                     start=True, stop=True)
                    nc.scalar.activation(out=exp_sb[p][:, sl], in_=pat[:],
                                         func=AF.Exp, scale=float(SCALER))
            if 1 <= i <= NT:
                t = i - 1
                sl = slice(t * TS, (t + 1) * TS)
                pden = ps.tile([128, TS], F32, tag="px", name="pden")
                for p in range(2):
                    nc.tensor.matmul(pden[:], denmask[p][:],
                                     exp_sb[p][:, sl],
                                     start=(p == 0), stop=(p == 1))
                nc.vector.reciprocal_approx_fast(out=rec_rep[:, sl],
                                                 in_=pden[:])
            if 2 <= i <= NT + 1:
                t = i - 2
                sl = slice(t * TS, (t + 1) * TS)
                pav = ps.tile([128, TS], F32, tag="px", name="pav")
                nc.tensor.matmul(pav[:], av0[:], exp_sb[0][:, sl],
                                 start=True, stop=False)
                nc.tensor.matmul(pav[:], av1[:], exp_sb[1][:, sl],
                                 start=False, stop=True)
                with nc.allow_low_precision(reason="attn out fits fp16"):
                    nc.vector.scalar_tensor_tensor(
                        out=cat_lo[:, sl], in0=pav[:], scalar=1.0,
                        in1=rec_rep[:, sl], op0=OP.mult, op1=OP.mult)
            if 3 <= i:
                t = i - 3
                sl = slice(t * TS, (t + 1) * TS)
                for m in range(2):
                    pp = ps.tile([128, TS], F32, tag="py", name="pp")
                    nc.tensor.matmul(pp[:],
                                     wproj[0][:, m * 128:(m + 1) * 128],
                                     cat_hi[:, sl], start=True, stop=False)
                    nc.tensor.matmul(pp[:],
                                     wproj[1][:, m * 128:(m + 1) * 128],
                                     cat_lo[:, sl], start=False, stop=True)
                    o_t = sm.tile([128, TS], F16, tag=f"o_t{m}",
                                  name=f"o_t{m}")
                    if m == 0:
                        nc.scalar.copy(out=o_t[:], in_=pp[:])
                    else:
                        nc.vector.tensor_copy(out=o_t[:], in_=pp[:])
                    nc.sync.dma_start(out=d["out"][b, m, :, sl], in_=o_t)
        if pooled_next is not None:
            pool_finish(b + 1, pooled_next)

    # ---------------- program ----------------
    x_cur = load_x(0)
    pooled = pool_reduce(x_cur)
    for t in range(NT):
        qkv_tile(0, t, x_cur)
        gq_tile(0, t, x_cur)
    pool_finish(0, pooled)

    for b in range(n_img):
        if b + 1 < n_img:
            x_next = load_x(b + 1)
            pooled_next = pool_reduce(x_next)
        else:
            x_next = pooled_next = None
        loop_a(b)
        loop_b(b, x_next, pooled_next)


def _build(n_img=B):
    nc = bacc.Bacc("TRN2", target_bir_lowering=False, debug=False,
                   num_devices=N_CORES)
    dt = nc.dram_tensor
    d = {
        "x": dt("x", [B, 2, 128, HW], F16, kind="ExternalInput").ap(),
        "wf16": dt("wf16", [128, W16_COLS], F16, kind="ExternalInput").ap(),
        "wf32": dt("wf32", [128, W32_COLS], F32, kind="ExternalInput").ap(),
        "wf8": dt("wf8", [128, W8_COLS], F8, kind="ExternalInput").ap(),
        "out": dt("out", [B, 2, 128, HW], F16, kind="ExternalOutput").ap(),
    }
    with tile.TileContext(nc) as tc, ExitStack() as ctx:
        _body(ctx, tc, d, n_img=n_img)
    nc.compile()
    return nc


_NC = None


def _prep_weights(qkv_w, dw_w, dw_b, act1_w, act1_b, act2_w, act2_b,
                  gq_w, gkv_w, proj_w):
    f32 = np.float32
    f16 = np.float16
    sc = np.float32(HEAD_DIM ** -0.5)

    wqkv = qkv_w.T.reshape(2, 128, 384).astype(f16)
    taps = dw_w.reshape(384, 9)            # [c, tap]
    idx = np.arange(128)
    # f16 diag blocks for the v branch only
    dwd = np.zeros((9, 128, 128), dtype=f16)
    for tp in range(9):
        dwd[tp, idx, idx] = taps[256:384, tp]
    dwdiag = dwd.transpose(1, 0, 2).reshape(128, 9 * 128)
    # fp8 DoubleRow pair blocks for q and k branches
    f8 = np.dtype(np.float32)  # placeholder; real cast below
    import ml_dtypes
    e4 = ml_dtypes.float8_e4m3
    wf8 = np.zeros((128, W8_COLS), dtype=e4)
    for br in range(2):
        tb = taps[128 * br:128 * (br + 1)]
        for pr, (ta_, tb_) in enumerate(DW_PAIRS):
            blk = np.zeros((128, 2, 128), np.float32)
            blk[idx, 0, idx] = tb[:, ta_]
            blk[idx, 1, idx] = tb[:, tb_]
            wf8[:, br * W8_BRANCH + pr * 256:
                br * W8_BRANCH + (pr + 1) * 256] = (
                blk.reshape(128, 256).astype(e4))
        t8 = np.zeros((128, 128), np.float32)
        t8[idx, idx] = tb[:, 8]
        wf8[:, br * W8_BRANCH + 1024:br * W8_BRANCH + 1152] = t8.astype(e4)
    wact1 = (act1_w * sc).T.astype(f16)
    wact2 = (act2_w / 6.0).T.astype(f16)
    wgq = gq_w.T.reshape(2, 128, 128).astype(f16)
    wgkv = (gkv_w / 49.0).T.reshape(2, 128, 256).astype(f16)
    wproj = proj_w.T.reshape(2, 128, 256).astype(f16)
    dm = np.zeros((2, 128, 128), dtype=f16)
    for p in range(2):
        for hl in range(2):
            head = 2 * p + hl
            dm[p, 64 * hl:64 * hl + 64, 32 * head:32 * head + 32] = 1.0

    blocks = {"wqkv0": wqkv[0], "wqkv1": wqkv[1], "dwdiag": dwdiag,
              "wact1": wact1, "wact2": wact2, "wgq0": wgq[0],
              "wgq1": wgq[1], "wgkv0": wgkv[0], "wgkv1": wgkv[1],
              "wproj0": wproj[0], "wproj1": wproj[1],
              "denmask0": dm[0], "denmask1": dm[1]}
    wf16 = np.zeros((128, W16_COLS), dtype=f16)
    for nm, (a, b_) in WCOL.items():
        wf16[:, a:b_] = blocks[nm]

    wf32 = np.zeros((128, W32_COLS), dtype=f32)
    wf32[:, 0:3] = dw_b.reshape(3, 128).T
    wf32[:, 3] = act1_b.astype(f32)
    wf32[:, 4] = act2_b.astype(f32)
    wf32[:, 5:14] = taps[256:384].astype(f32)   # dw-v taps for DVE path

    return {"wf16": np.ascontiguousarray(wf16),
            "wf32": np.ascontiguousarray(wf32),
            "wf8": np.ascontiguousarray(wf8)}


def kernel(**inputs):
    global _NC
    x = inputs["x"]
    w = _prep_weights(
        inputs["qkv_w"], inputs["dw_w"], inputs["dw_b"],
        inputs["act1_w"], inputs["act1_b"], inputs["act2_w"],
        inputs["act2_b"], inputs["gq_w"], inputs["gkv_w"], inputs["proj_w"])
    if _NC is None:
        _NC = _build()
    in_maps = []
    for core in range(N_CORES):
        m = dict(w)
        m["x"] = np.ascontiguousarray(
            x[core * B:(core + 1) * B].reshape(B, 2, 128, HW)
            .astype(np.float16))
        in_maps.append(m)
    res = run_bass_kernel_spmd(_NC, in_maps, core_ids=list(range(N_CORES)))
    out = np.concatenate([r["out"] for r in res.results], axis=0)
    return out.reshape(B_FULL, C, H, W).astype(np.float32)
